# revision 41
# baseline (speedup 1.0000x reference)
"""Trainium2 Bass kernel for the GRU autoencoder (v8).

Distribution (8 NeuronCores):
  Encode : chain-parallel x batch-parallel. Core j handles GRU chain j//2
           (xf, xb, ef, eb) on batch half j%2 (128 rows), uniform 100-step
           loop; the 50-step x-chains get 50 exact identity steps (z forced
           to 1 via a +BIG flag row). AllToAll reshards 16-row slices so each
           core decodes global rows [16j:16j+16] u [128+16j:+16].

v8 (from 2.55ms baseline to 1.24ms): everything bf16 end-to-end (weights
shipped as bf16 from the host — no device-side casting, lazy weight DMA
spread across encode steps), encoder eltwise tail reordered/halved with
filler transposes + pre-issued input-side matmuls to hold HAM at K=8/8,
decoder fully transposed (weights-stationary, [feature, batch] layout,
N=32 moving matmuls, zero per-step transposes, biases as K=1 stationary-row
matmuls, relus on the DVE).
"""

import sys

sys.path.insert(0, "/opt/trn_rl_repo")

import ml_dtypes
import numpy as np

import concourse.bass as bass
import concourse.mybir as mybir
import concourse.tile as tile
from concourse import bacc
from concourse.masks import make_identity

dt = mybir.dt
AF = mybir.ActivationFunctionType
OP = mybir.AluOpType

B, TX, TY, NX, NY, H, HOR = 256, 50, 100, 64, 64, 512, 60
M1, M2 = 1024, 512
G = 3 * H
NCORE = 8
BE = 128   # encoder batch rows per core
BD = 32    # decoder batch rows per core
BIG = 30000.0

F32, BF16 = dt.float32, dt.bfloat16
BF = ml_dtypes.bfloat16


def build_nc(et=100, hor=60):
    nc = bacc.Bacc("TRN2", target_bir_lowering=False, debug=False,
                   num_devices=NCORE)

    # ---- DRAM parameters (bf16 except the ACT bias column) ----
    d_xin = nc.dram_tensor("xin", [66, et * BE], BF16, kind="ExternalInput")
    d_wih = nc.dram_tensor("wih_aug", [66, G], BF16, kind="ExternalInput")
    d_whh = nc.dram_tensor("whh_t", [H, G], BF16, kind="ExternalInput")
    d_bhhn = nc.dram_tensor("bhhn_row", [1, H], BF16, kind="ExternalInput")

    d_em1 = nc.dram_tensor("em_w1t", [2 * H, M1], BF16, kind="ExternalInput")
    d_em2 = nc.dram_tensor("em_w2t", [M1, M2], BF16, kind="ExternalInput")
    d_eow = nc.dram_tensor("eo_wt", [M2, H], BF16, kind="ExternalInput")
    d_dcw = nc.dram_tensor("dc_wt", [2 * H, G], BF16, kind="ExternalInput")
    d_midb = nc.dram_tensor("mid_bias", [1, 3584], BF16,
                            kind="ExternalInput")

    d_dwy = nc.dram_tensor("dwy_t", [NY, G], BF16, kind="ExternalInput")
    d_dwhh = nc.dram_tensor("dwhh_t", [H, G], BF16, kind="ExternalInput")
    d_dbhhn = nc.dram_tensor("dbhhn_row", [1, H], BF16, kind="ExternalInput")
    d_dm1 = nc.dram_tensor("dm_w1t", [H, M1], BF16, kind="ExternalInput")
    d_dm1b = nc.dram_tensor("dm_b1row", [1, M1], BF16, kind="ExternalInput")
    d_dm2 = nc.dram_tensor("dm_w2t", [M1, M2], BF16, kind="ExternalInput")
    d_dm2b = nc.dram_tensor("dm_b2row", [1, M2], BF16, kind="ExternalInput")
    d_dow = nc.dram_tensor("do_wt", [M2, NY], BF16, kind="ExternalInput")
    d_dobr = nc.dram_tensor("do_brow", [1, NY], BF16, kind="ExternalInput")
    d_xlast = nc.dram_tensor("xlast_t", [NX, BD], BF16, kind="ExternalInput")

    d_out = nc.dram_tensor("out", [hor * NY, BD], F32, kind="ExternalOutput")

    cc_in = nc.dram_tensor("cc_in", [BE, H], BF16)
    cc_out = nc.dram_tensor("cc_out", [NCORE, 16, H], BF16)

    with tile.TileContext(nc) as tc:
        with tc.tile_pool(name="pe", bufs=1) as pe, \
             tc.tile_pool(name="wts", bufs=1) as wts, \
             tc.tile_pool(name="xsp", bufs=2) as xsp, \
             tc.tile_pool(name="st", bufs=2) as st, \
             tc.tile_pool(name="tp", bufs=2) as tp, \
             tc.tile_pool(name="md", bufs=1) as md, \
             tc.tile_pool(name="pA", bufs=2, space="PSUM") as pA, \
             tc.tile_pool(name="pB", bufs=2, space="PSUM") as pB, \
             tc.tile_pool(name="pC", bufs=2, space="PSUM") as pC, \
             tc.tile_pool(name="pD", bufs=1, space="PSUM") as pD, \
             tc.tile_pool(name="pTR", bufs=1, space="PSUM") as pTR:

            # ---------- constants ----------
            idf = pe.tile([128, 128], F32, tag="idf")
            make_identity(nc, idf[:])
            idb = pe.tile([128, 128], BF16, tag="idb")
            nc.gpsimd.tensor_copy(idb[:], idf[:])
            ones_b = pe.tile([1, 128], BF16, tag="ones_b")
            nc.gpsimd.memset(ones_b[:], 1.0)
            zero_b = pe.tile([128, 512], BF16, tag="zero_b")
            nc.gpsimd.memset(zero_b[:], 0.0)

            def load_direct(pool, dram_ap, rows, cols, tag):
                r = pool.tile([rows, cols], BF16, tag=tag)
                nc.sync.dma_start(r[:], dram_ap)
                return r

            # Middle/decoder weights: allocate now, DMA lazily inside the
            # encode loop (one tile per step) so the startup xin load isn't
            # queued behind ~6MB of weight traffic.
            wload = []

            def load_lazy(dram_ap, rows, cols, tag, rdt=BF16):
                r = wts.tile([rows, cols], rdt, tag=tag)
                wload.append((r, dram_ap))
                return r

            # ---------- encoder weights (needed immediately) ----------
            wih_b = load_direct(wts, d_wih[:], 66, G, "wih")
            whh_b = [load_direct(wts, d_whh[128 * c:128 * (c + 1), :],
                                 128, G, f"whh{c}") for c in range(4)]
            ebhhn = load_direct(wts, d_bhhn[:], 1, H, "ebhhn")

            # ---------- encoder state ----------
            hT = pe.tile([128, H], BF16, tag="hT0")       # [feat%128, 4x128b]
            nc.vector.tensor_copy(hT[:], zero_b[:])
            h_bh = pe.tile([BE, H], BF16, tag="h0")       # [batch, feat]
            nc.gpsimd.memset(h_bh[:], 0.0)

            # ---------- middle + decoder weights (lazy bf16 DMA) ----------
            em1_b = [load_lazy(d_em1[128 * c:128 * (c + 1), :],
                               128, M1, f"em1_{c}") for c in range(8)]
            em2_b = [load_lazy(d_em2[128 * c:128 * (c + 1), :],
                               128, M2, f"em2_{c}") for c in range(8)]
            eo_b = [load_lazy(d_eow[128 * c:128 * (c + 1), :],
                              128, H, f"eo_{c}") for c in range(4)]
            dcw_b = [load_lazy(d_dcw[128 * c:128 * (c + 1), :],
                               128, G, f"dcw_{c}") for c in range(8)]
            bias_b = pe.tile([1, 3584], BF16, tag="bias_b")
            wload.append((bias_b, d_midb[:]))

            dwhh_b = [load_lazy(d_dwhh[128 * c:128 * (c + 1), :],
                                128, G, f"dwhh{c}") for c in range(4)]
            dm1_b = [load_lazy(d_dm1[128 * c:128 * (c + 1), :],
                               128, M1, f"dm1_{c}") for c in range(4)]
            dm2_b = [load_lazy(d_dm2[128 * c:128 * (c + 1), :],
                               128, M2, f"dm2_{c}") for c in range(8)]
            dow_b = [load_lazy(d_dow[128 * c:128 * (c + 1), :],
                               128, NY, f"dow_{c}") for c in range(4)]
            dbhhn_s = load_lazy(d_dbhhn[:], 1, H, "dbhhn")
            dm1b_s = load_lazy(d_dm1b[:], 1, M1, "dm1b")
            dm2b_s = load_lazy(d_dm2b[:], 1, M2, "dm2b")
            dob_r = load_direct(wts, d_dobr[:], 1, NY, "dobr")
            # ycw: rows 0:64 = Wy^T, rows 64:96 = const (filled post-middle).
            ycw = pe.tile([96, G], BF16, tag="ycw")
            wload.append((ycw[0:NY, :], d_dwy[:]))
            # ypc: rows 0:64 = y_t, rows 64:96 = I32 (selects const rows).
            ypc = pe.tile([96, BD], BF16, tag="ypc")
            nc.sync.dma_start(ypc[0:NX, :], d_xlast[:])
            nc.gpsimd.tensor_copy(ypc[64:96, :], idb[0:32, 0:32])

            # =======================================================
            # Encode loop, software-pipelined in feature halves.
            # =======================================================
            def enc_alloc():
                ga = pA.tile([BE, 512], F32, tag="A")
                gb = pB.tile([BE, 512], F32, tag="B")
                gc = pC.tile([BE, 512], F32, tag="C")
                gd = pD.tile([BE, 512], F32, tag="D")
                return ga, gb, gc, gd

            def enc_xs_mms(xs, ga, gb, gc):
                nc.tensor.matmul(ga[:], xs[:], wih_b[:, 0:512],
                                 start=True, stop=False)
                nc.tensor.matmul(gb[:], xs[:], wih_b[:, 512:1024],
                                 start=True, stop=False)
                nc.tensor.matmul(gc[:], xs[:], wih_b[:, 1024:1536],
                                 start=True, stop=True)

            def enc_bias_mm(gd):
                nc.tensor.matmul(gd[:], ones_b[0:1, 0:BE], ebhhn[:],
                                 start=True, stop=False)

            def load_xs(t):
                xb = xsp.tile([66, 128], BF16, tag="xs_b")
                nc.sync.dma_start(xb[:], d_xin[:, t * BE:(t + 1) * BE])
                return xb

            xs = load_xs(0)
            ga, gb, gc, gd = enc_alloc()
            enc_xs_mms(xs, ga, gb, gc)
            enc_bias_mm(gd)

            for t in range(et):
                last = (t == et - 1)
                # h-side matmuls, bank-major: r-gates, z-gates, n-h-gates
                # (z early so b=z*h and omz are off the critical path).
                for c in range(4):
                    nc.tensor.matmul(ga[:], hT[:, 128 * c:128 * (c + 1)],
                                     whh_b[c][:, 0:512],
                                     start=False, stop=(c == 3))
                for c in range(4):
                    nc.tensor.matmul(gb[:], hT[:, 128 * c:128 * (c + 1)],
                                     whh_b[c][:, 512:1024],
                                     start=False, stop=(c == 3))
                for c in range(4):
                    nc.tensor.matmul(gd[:], hT[:, 128 * c:128 * (c + 1)],
                                     whh_b[c][:, 1024:1536],
                                     start=False, stop=(c == 3))
                if not last:
                    xs_n = load_xs(t + 1)
                    if t < len(wload):
                        wa, wd = wload[t]
                        nc.sync.dma_start(wa[:], wd)
                    ga_n, gb_n, gc_n, gd_n = enc_alloc()

                # ---- eltwise: h' = (1-z)*n + z*h; rhn/npre and the tail
                # run in feature halves so tanh/hT chunks land early; dummy
                # transposes chained on eltwise temps keep the PE active
                # through the tail (HAM stays at K=8/8).
                r_t = tp.tile([BE, 512], BF16, tag="r")
                z_t = tp.tile([BE, 512], BF16, tag="z")
                n_t = tp.tile([BE, 512], BF16, tag="n")
                rhn = tp.tile([BE, 512], BF16, tag="rhn")
                npre = tp.tile([BE, 512], BF16, tag="npre")
                omz = tp.tile([BE, 512], BF16, tag="omz")
                b_t = tp.tile([BE, 512], BF16, tag="b")
                a_t = tp.tile([BE, 512], BF16, tag="a")
                h_new = st.tile([BE, H], BF16, tag="h")
                ptr = pTR.tile([128, 512], BF16, tag="TR")
                hT_new = st.tile([128, H], BF16, tag="hT")

                sl = [slice(0, 256), slice(256, 512)]
                nc.scalar.activation(r_t[:], ga[:], AF.Sigmoid)
                nc.scalar.activation(z_t[:], gb[:], AF.Sigmoid)
                nc.vector.tensor_scalar(omz[:], z_t[:], -1.0, 1.0,
                                        OP.mult, OP.add)
                nc.gpsimd.tensor_mul(b_t[:, sl[0]], z_t[:, sl[0]],
                                     h_bh[:, sl[0]])
                nc.gpsimd.tensor_mul(b_t[:, sl[1]], z_t[:, sl[1]],
                                     h_bh[:, sl[1]])
                for s in range(2):
                    nc.vector.tensor_mul(rhn[:, sl[s]], r_t[:, sl[s]],
                                         gd[:, sl[s]])
                    nc.vector.tensor_add(npre[:, sl[s]], rhn[:, sl[s]],
                                         gc[:, sl[s]])
                    nc.scalar.activation(n_t[:, sl[s]], npre[:, sl[s]],
                                         AF.Tanh)
                if not last:
                    # Interleave the (data-independent) next-step xs matmuls
                    # with filler transposes gated on eltwise temps so the
                    # PE keeps a high duty cycle through the tail and HAM
                    # stays at K=8/8 into the next gate burst.
                    nc.tensor.transpose(ptr[:, 0:128], r_t[:, 0:128],
                                        idb[:])
                    nc.tensor.transpose(ptr[:, 128:256], r_t[:, 128:256],
                                        idb[:])
                    nc.tensor.matmul(ga_n[:], xs_n[:], wih_b[:, 0:512],
                                     start=True, stop=False)
                    nc.tensor.transpose(ptr[:, 256:384], npre[:, 0:128],
                                        idb[:])
                    nc.tensor.transpose(ptr[:, 384:512], npre[:, 128:256],
                                        idb[:])
                    nc.tensor.matmul(gb_n[:], xs_n[:], wih_b[:, 512:1024],
                                     start=True, stop=False)
                    nc.tensor.transpose(ptr[:, 0:128], n_t[:, 0:128],
                                        idb[:])
                    nc.tensor.transpose(ptr[:, 128:256], n_t[:, 128:256],
                                        idb[:])
                    nc.tensor.matmul(gc_n[:], xs_n[:], wih_b[:, 1024:1536],
                                     start=True, stop=True)
                for s in range(2):
                    nc.vector.tensor_mul(a_t[:, sl[s]], omz[:, sl[s]],
                                         n_t[:, sl[s]])
                    nc.vector.tensor_add(h_new[:, sl[s]], a_t[:, sl[s]],
                                         b_t[:, sl[s]])
                    if not last:
                        for c in (2 * s, 2 * s + 1):
                            nc.tensor.transpose(
                                ptr[:, 128 * c:128 * (c + 1)],
                                h_new[:, 128 * c:128 * (c + 1)], idb[:])
                        if s == 0:
                            nc.scalar.copy(hT_new[:, sl[0]], ptr[:, sl[0]])
                        else:
                            nc.vector.tensor_copy(hT_new[:, sl[1]],
                                                  ptr[:, sl[1]])
                if not last:
                    enc_bias_mm(gd_n)
                    hT = hT_new
                    ga, gb, gc, gd = ga_n, gb_n, gc_n, gd_n
                h_bh = h_new

            # ---------- reshard: AllToAll of 16-row slices (bf16) ----------
            nc.sync.dma_start(cc_in[:], h_bh[:])
            nc.gpsimd.collective_compute(
                "AllToAll", OP.bypass,
                replica_groups=[list(range(NCORE))],
                ins=[cc_in[:]], outs=[cc_out[:]])

            pxa = md.tile([BD, H], BF16, tag="pA")
            pxb = md.tile([BD, H], BF16, tag="pB")
            pya = md.tile([BD, H], BF16, tag="pA")
            pyb = md.tile([BD, H], BF16, tag="pB")
            nc.sync.dma_start(pxa[0:16, :], cc_out[0][:])
            nc.sync.dma_start(pxa[16:32, :], cc_out[1][:])
            nc.sync.dma_start(pxb[0:16, :], cc_out[2][:])
            nc.sync.dma_start(pxb[16:32, :], cc_out[3][:])
            nc.sync.dma_start(pya[0:16, :], cc_out[4][:])
            nc.sync.dma_start(pya[16:32, :], cc_out[5][:])
            nc.sync.dma_start(pyb[0:16, :], cc_out[6][:])
            nc.sync.dma_start(pyb[16:32, :], cc_out[7][:])
            # PE warmup off the collective outputs: a dense burst of
            # transposes re-engages HAM before the middle MLP runs.
            pwu = pTR.tile([128, 512], BF16, tag="TR")
            for i, src in enumerate((pxa, pxb, pya, pyb)):
                for c in range(4):
                    nc.tensor.transpose(
                        pwu[:, (4 * i + c) * 32:(4 * i + c + 1) * 32],
                        src[:, 128 * c:128 * (c + 1)], idb[0:32, 0:32])
            hx = md.tile([BD, H], F32, tag="hx")
            hy = md.tile([BD, H], F32, tag="hy")
            nc.vector.tensor_add(hx[:], pxa[:], pxb[:])
            nc.vector.tensor_add(hy[:], pya[:], pyb[:])

            def trsp_b(src, cols, tag):
                """src [BD, cols] f32 -> bf16 [128, (cols//128)*BD] via PE."""
                nch = cols // 128
                p = pTR.tile([128, 512], F32, tag="TR")
                for c in range(nch):
                    nc.tensor.transpose(p[:, BD * c:BD * (c + 1)],
                                        src[:, 128 * c:128 * (c + 1)],
                                        idf[0:32, 0:32])
                o = md.tile([128, nch * BD], BF16, tag=tag)
                nc.scalar.copy(o[:], p[:, 0:nch * BD])
                return o

            hxT = trsp_b(hx, H, "hxT")
            hyT = trsp_b(hy, H, "hyT")

            # ---- middle MLP (batch-major, activations stationary) ----
            m1a = pA.tile([BD, 512], F32, tag="A")
            m1b = pB.tile([BD, 512], F32, tag="B")
            for c in range(8):
                wt = em1_b[c]
                s = (hxT if c < 4 else hyT)[:, BD * (c % 4):BD * (c % 4 + 1)]
                nc.tensor.matmul(m1a[:], s, wt[:, 0:512],
                                 start=(c == 0), stop=False)
                nc.tensor.matmul(m1b[:], s, wt[:, 512:1024],
                                 start=(c == 0), stop=False)
            nc.tensor.matmul(m1a[:], ones_b[0:1, 0:BD],
                             bias_b[0:1, 1536:2048], start=False, stop=True)
            nc.tensor.matmul(m1b[:], ones_b[0:1, 0:BD],
                             bias_b[0:1, 2048:2560], start=False, stop=True)
            hm1 = md.tile([BD, M1], F32, tag="hm1")
            nc.scalar.activation(hm1[:, 0:512], m1a[:], AF.Relu)
            nc.scalar.activation(hm1[:, 512:1024], m1b[:], AF.Relu)
            hm1T = trsp_b(hm1, M1, "hm1T_m")

            m2 = pC.tile([BD, M2], F32, tag="C")
            for c in range(8):
                nc.tensor.matmul(m2[:], hm1T[:, BD * c:BD * (c + 1)],
                                 em2_b[c][:], start=(c == 0), stop=False)
            nc.tensor.matmul(m2[:], ones_b[0:1, 0:BD], bias_b[0:1, 2560:3072],
                             start=False, stop=True)
            hm2 = md.tile([BD, M2], F32, tag="hm2")
            nc.scalar.activation(hm2[:], m2[:], AF.Relu)
            hm2T = trsp_b(hm2, M2, "hm2T_m")

            zp = pD.tile([BD, 512], F32, tag="D")
            for c in range(4):
                nc.tensor.matmul(zp[:, 0:H], hm2T[:, BD * c:BD * (c + 1)],
                                 eo_b[c][:], start=(c == 0), stop=False)
            nc.tensor.matmul(zp[:, 0:H], ones_b[0:1, 0:BD],
                             bias_b[0:1, 3072:3584], start=False, stop=True)
            z_sb = md.tile([BD, H], F32, tag="z_sb")
            nc.scalar.copy(z_sb[:], zp[:, 0:H])
            zT = trsp_b(z_sb, H, "zT")

            # const = cat(h_x, z) @ d_Wih[:, :2H].T + d_bih + d_bhh(r,z)
            cpa = pA.tile([BD, 512], F32, tag="A")
            cpb = pB.tile([BD, 512], F32, tag="B")
            cpn = pC.tile([BD, 512], F32, tag="C")
            for c in range(8):
                wt = dcw_b[c]
                s = (hxT if c < 4 else zT)[:, BD * (c % 4):BD * (c % 4 + 1)]
                nc.tensor.matmul(cpa[:], s, wt[:, 0:512],
                                 start=(c == 0), stop=False)
                nc.tensor.matmul(cpb[:], s, wt[:, 512:1024],
                                 start=(c == 0), stop=False)
                nc.tensor.matmul(cpn[:], s, wt[:, 1024:1536],
                                 start=(c == 0), stop=False)
            nc.tensor.matmul(cpa[:], ones_b[0:1, 0:BD],
                             bias_b[0:1, 0:512], start=False, stop=True)
            nc.tensor.matmul(cpb[:], ones_b[0:1, 0:BD],
                             bias_b[0:1, 512:1024], start=False, stop=True)
            nc.tensor.matmul(cpn[:], ones_b[0:1, 0:BD],
                             bias_b[0:1, 1024:1536], start=False, stop=True)
            nc.vector.tensor_copy(ycw[64:96, 0:512], cpa[:])
            nc.vector.tensor_copy(ycw[64:96, 512:1024], cpb[:])
            nc.vector.tensor_copy(ycw[64:96, 1024:1536], cpn[:])

            # =======================================================
            # Decode loop: fully transposed, h as [128, 4*32] bf16.
            # =======================================================
            hbf = st.tile([128, 4 * BD], BF16, tag="hbf")
            nc.gpsimd.memset(hbf[:], 0.0)

            def dec_whh(rz, hgn, hbf_src):
                for j in range(8):
                    for k in range(4):
                        nc.tensor.matmul(
                            rz[:, BD * j:BD * (j + 1)],
                            dwhh_b[k][:, 128 * j:128 * (j + 1)],
                            hbf_src[:, BD * k:BD * (k + 1)],
                            start=(j == 0 and k == 0), stop=False)
                for j in range(4):
                    for k in range(4):
                        nc.tensor.matmul(
                            hgn[:, BD * j:BD * (j + 1)],
                            dwhh_b[k][:, 128 * (8 + j):128 * (9 + j)],
                            hbf_src[:, BD * k:BD * (k + 1)],
                            start=(j == 0 and k == 0), stop=False)
                    nc.tensor.matmul(hgn[:, BD * j:BD * (j + 1)],
                                     dbhhn_s[0:1, 128 * j:128 * (j + 1)],
                                     ones_b[0:1, 0:BD],
                                     start=False, stop=(j == 3))

            def dec_m1bias(m1):
                for j in range(8):
                    nc.tensor.matmul(m1[:, BD * j:BD * (j + 1)],
                                     dm1b_s[0:1, 128 * j:128 * (j + 1)],
                                     ones_b[0:1, 0:BD],
                                     start=(j == 0), stop=False)

            def dec_m2bias(m2d):
                for j in range(4):
                    nc.tensor.matmul(m2d[:, BD * j:BD * (j + 1)],
                                     dm2b_s[0:1, 128 * j:128 * (j + 1)],
                                     ones_b[0:1, 0:BD],
                                     start=(j == 0), stop=False)

            rz = pA.tile([128, 512], F32, tag="A")
            hgn = pB.tile([128, 512], F32, tag="B")
            m1 = pD.tile([128, 512], F32, tag="D")
            m2d = pTR.tile([128, 512], F32, tag="TR")
            dec_m1bias(m1)
            dec_m2bias(m2d)
            # t=0: h=0, so no Whh matmuls; hgn(0) = bias only.
            for j in range(4):
                nc.tensor.matmul(hgn[:, BD * j:BD * (j + 1)],
                                 dbhhn_s[0:1, 128 * j:128 * (j + 1)],
                                 ones_b[0:1, 0:BD],
                                 start=(j == 0), stop=(j == 3))

            for t in range(hor):
                lastd = (t == hor - 1)
                # ---- y/const-side gate matmuls ----
                an = pC.tile([128, 512], F32, tag="C")
                for j in range(8):
                    nc.tensor.matmul(rz[:, BD * j:BD * (j + 1)],
                                     ycw[:, 128 * j:128 * (j + 1)], ypc[:],
                                     start=(t == 0 and j == 0),
                                     stop=(j == 7))
                for j in range(4):
                    nc.tensor.matmul(an[:, BD * j:BD * (j + 1)],
                                     ycw[:, 128 * (8 + j):128 * (9 + j)],
                                     ypc[:], start=(j == 0), stop=(j == 3))

                # ---- GRU eltwise, transposed layout [128, 4*32]; tail in
                # chunk-pair halves so M1 starts on half 0 early ----
                # one ACT op covers both r (cols 0:128) and z (128:256)
                rz_s = tp.tile([128, 8 * BD], BF16, tag="drz")
                nc.scalar.activation(rz_s[:], rz[:, 0:8 * BD], AF.Sigmoid)
                r_t = rz_s[:, 0:4 * BD]
                z_t = rz_s[:, 4 * BD:8 * BD]
                omz = tp.tile([128, 4 * BD], BF16, tag="domz")
                nc.vector.tensor_scalar(omz[:], z_t[:], -1.0, 1.0,
                                        OP.mult, OP.add)
                u_t = tp.tile([128, 4 * BD], BF16, tag="du")
                nc.gpsimd.tensor_mul(u_t[:], z_t[:], hbf[:])
                t2 = tp.tile([128, 4 * BD], BF16, tag="dt2")
                npre = tp.tile([128, 4 * BD], BF16, tag="dnp")
                n_t = tp.tile([128, 4 * BD], BF16, tag="dn")
                a_t = tp.tile([128, 4 * BD], BF16, tag="da")
                hbf_n = st.tile([128, 4 * BD], BF16, tag="hbf")
                for s in range(2):
                    dsl = slice(2 * BD * s, 2 * BD * (s + 1))
                    nc.vector.tensor_mul(t2[:, dsl], r_t[:, dsl],
                                         hgn[:, dsl])
                    nc.vector.tensor_add(npre[:, dsl], t2[:, dsl],
                                         an[:, dsl])
                    nc.scalar.activation(n_t[:, dsl], npre[:, dsl], AF.Tanh)
                    nc.vector.tensor_mul(a_t[:, dsl], omz[:, dsl],
                                         n_t[:, dsl])
                    nc.vector.tensor_add(hbf_n[:, dsl], a_t[:, dsl],
                                         u_t[:, dsl])
                    # M1 on the two ready h chunks
                    for k in (2 * s, 2 * s + 1):
                        for j in range(8):
                            nc.tensor.matmul(
                                m1[:, BD * j:BD * (j + 1)],
                                dm1_b[k][:, 128 * j:128 * (j + 1)],
                                hbf_n[:, BD * k:BD * (k + 1)],
                                start=False, stop=(k == 3 and j == 7))
                hbf = hbf_n
                hm1_s = tp.tile([128, 8 * BD], BF16, tag="dhm1")
                # ---- M2, with relu1 split so the first half starts early --
                for sh in range(2):
                    rsl = slice(4 * BD * sh, 4 * BD * (sh + 1))
                    nc.vector.tensor_scalar_max(hm1_s[:, rsl], m1[:, rsl],
                                                0.0)
                    for k in range(4 * sh, 4 * sh + 4):
                        for j in range(4):
                            nc.tensor.matmul(
                                m2d[:, BD * j:BD * (j + 1)],
                                dm2_b[k][:, 128 * j:128 * (j + 1)],
                                hm1_s[:, BD * k:BD * (k + 1)],
                                start=False, stop=(k == 7 and j == 3))
                hm2_s = tp.tile([128, 4 * BD], BF16, tag="dhm2")
                nc.vector.tensor_scalar_max(hm2_s[:], m2d[:, 0:4 * BD], 0.0)

                # bias pre-issue for t+1 (fills the out-matmul wait)
                if not lastd:
                    m1_n = pD.tile([128, 512], F32, tag="D")
                    dec_m1bias(m1_n)
                    m2_n = pTR.tile([128, 512], F32, tag="TR")
                    dec_m2bias(m2_n)

                # ---- output head: y [64, 32]; do_b rides as a K=1 matmul --
                yb = pC.tile([128, 512], F32, tag="C")
                nc.tensor.matmul(yb[0:NY, 0:BD], dob_r[0:1, 0:NY],
                                 ones_b[0:1, 0:BD], start=True, stop=False)
                for k in range(4):
                    nc.tensor.matmul(yb[0:NY, 0:BD],
                                     dow_b[k][:, 0:NY],
                                     hm2_s[:, BD * k:BD * (k + 1)],
                                     start=False, stop=(k == 3))
                if not lastd:
                    # critical path: feed y back (bf16) before the f32 copy
                    nc.vector.tensor_copy(ypc[0:NY, :], yb[0:NY, 0:BD])
                y_f = tp.tile([NY, BD], F32, tag="dy")
                nc.scalar.copy(y_f[:], yb[0:NY, 0:BD])
                nc.sync.dma_start(d_out[NY * t:NY * (t + 1), :], y_f[:])
                if not lastd:
                    # pre-issue next step's h-side matmuls
                    rz_n = pA.tile([128, 512], F32, tag="A")
                    hgn_n = pB.tile([128, 512], F32, tag="B")
                    dec_whh(rz_n, hgn_n, hbf)
                    rz, hgn, m1, m2d = rz_n, hgn_n, m1_n, m2_n

    nc.compile()
    return nc


# ---------------------------------------------------------------------------
# Host-side sharding
# ---------------------------------------------------------------------------

def shard_inputs(inp, et=100, hor=60):
    f32 = np.float32

    def bf(a):
        return np.ascontiguousarray(np.asarray(a, f32).astype(BF))

    x, y = np.asarray(inp["x"], f32), np.asarray(inp["y"], f32)
    chains = [("xf", False, x), ("xb", True, x),
              ("ef", False, y), ("eb", True, y)]
    in_maps = []
    shared = {}

    def wih_aug(pre):
        wih = np.asarray(inp[pre + "_Wih"], f32)
        bih = np.asarray(inp[pre + "_bih"], f32)
        bhh = np.asarray(inp[pre + "_bhh"], f32)
        aug = np.zeros((66, G), f32)
        aug[0:64, :] = wih.T
        bias = bih.copy()
        bias[0:2 * H] += bhh[0:2 * H]
        aug[64, :] = bias
        aug[65, H:2 * H] = BIG
        return bf(aug)

    d_Wih = np.asarray(inp["d_Wih"], f32)
    d_bih = np.asarray(inp["d_bih"], f32)
    d_bhh = np.asarray(inp["d_bhh"], f32)
    dc_b = d_bih.copy()
    dc_b[0:2 * H] += d_bhh[0:2 * H]

    shared["em_w1t"] = bf(np.asarray(inp["em_W1"], f32).T)
    shared["em_w2t"] = bf(np.asarray(inp["em_W2"], f32).T)
    shared["eo_wt"] = bf(np.asarray(inp["eo_W"], f32).T)
    shared["dc_wt"] = bf(d_Wih[:, 0:2 * H].T)
    midb = np.concatenate([dc_b, np.asarray(inp["em_b1"], f32),
                           np.asarray(inp["em_b2"], f32),
                           np.asarray(inp["eo_b"], f32)])[None, :]
    shared["mid_bias"] = bf(midb)
    shared["dwy_t"] = bf(d_Wih[:, 2 * H:].T)
    shared["dwhh_t"] = bf(np.asarray(inp["d_Whh"], f32).T)
    shared["dbhhn_row"] = bf(d_bhh[None, 2 * H:])
    shared["dm_w1t"] = bf(np.asarray(inp["dm_W1"], f32).T)
    shared["dm_b1row"] = bf(np.asarray(inp["dm_b1"], f32)[None, :])
    shared["dm_w2t"] = bf(np.asarray(inp["dm_W2"], f32).T)
    shared["dm_b2row"] = bf(np.asarray(inp["dm_b2"], f32)[None, :])
    shared["do_wt"] = bf(np.asarray(inp["do_W"], f32).T)
    shared["do_brow"] = bf(np.asarray(inp["do_b"], f32)[None, :])

    for j in range(NCORE):
        chain, half = j // 2, j % 2
        pre, rev, seq = chains[chain]
        T = seq.shape[1]
        s = seq[128 * half:128 * (half + 1)]          # [128, T, 64]
        xin = np.zeros((66, et, BE), f32)
        xin[64, :, :] = 1.0
        pad = et - T
        if pad:
            xin[65, 0:pad, :] = 1.0
        order = np.arange(T)[::-1] if rev else np.arange(T)
        xin[0:64, pad:, :] = s[:, order, :].transpose(2, 1, 0)
        m = dict(shared)
        m["xin"] = bf(xin.reshape(66, et * BE))
        m["wih_aug"] = wih_aug(pre)
        m["whh_t"] = bf(np.asarray(inp[pre + "_Whh"], f32).T)
        m["bhhn_row"] = bf(np.asarray(inp[pre + "_bhh"], f32)[None, 2 * H:])
        xl = np.concatenate([x[16 * j:16 * j + 16, -1, :],
                             x[128 + 16 * j:128 + 16 * j + 16, -1, :]])
        m["xlast_t"] = bf(xl.T)
        in_maps.append(m)
    return in_maps


def unshard(results, hor=60):
    out = np.zeros((B, hor, NY), np.float32)
    for j in range(NCORE):
        o = results[j]["out"].reshape(hor, NY, BD).transpose(2, 0, 1)
        out[16 * j:16 * j + 16] = o[0:16]
        out[128 + 16 * j:128 + 16 * j + 16] = o[16:32]
    return out


_NC = None


def kernel(**inputs):
    global _NC
    from concourse.bass_utils import run_bass_kernel_spmd
    if _NC is None:
        _NC = build_nc()
    in_maps = shard_inputs(inputs)
    res = run_bass_kernel_spmd(_NC, in_maps, core_ids=list(range(NCORE)))
    return unshard(res.results)


# revision 43
# speedup vs baseline: 1.0061x; 1.0061x over previous
"""Trainium2 Bass kernel for the GRU autoencoder (v8).

Distribution (8 NeuronCores):
  Encode : chain-parallel x batch-parallel. Core j handles GRU chain j//2
           (xf, xb, ef, eb) on batch half j%2 (128 rows), uniform 100-step
           loop; the 50-step x-chains get 50 exact identity steps (z forced
           to 1 via a +BIG flag row). AllToAll reshards 16-row slices so each
           core decodes global rows [16j:16j+16] u [128+16j:+16].

v8 (from 2.55ms baseline to 1.24ms): everything bf16 end-to-end (weights
shipped as bf16 from the host — no device-side casting, lazy weight DMA
spread across encode steps), encoder eltwise tail reordered/halved with
filler transposes + pre-issued input-side matmuls to hold HAM at K=8/8,
decoder fully transposed (weights-stationary, [feature, batch] layout,
N=32 moving matmuls, zero per-step transposes, biases as K=1 stationary-row
matmuls, relus on the DVE).
"""

import sys

sys.path.insert(0, "/opt/trn_rl_repo")

import ml_dtypes
import numpy as np

import concourse.bass as bass
import concourse.mybir as mybir
import concourse.tile as tile
from concourse import bacc
from concourse.masks import make_identity

dt = mybir.dt
AF = mybir.ActivationFunctionType
OP = mybir.AluOpType

B, TX, TY, NX, NY, H, HOR = 256, 50, 100, 64, 64, 512, 60
M1, M2 = 1024, 512
G = 3 * H
NCORE = 8
BE = 128   # encoder batch rows per core
BD = 32    # decoder batch rows per core
BIG = 30000.0

F32, BF16 = dt.float32, dt.bfloat16
BF = ml_dtypes.bfloat16


def build_nc(et=100, hor=60):
    nc = bacc.Bacc("TRN2", target_bir_lowering=False, debug=False,
                   num_devices=NCORE)

    # ---- DRAM parameters (bf16 except the ACT bias column) ----
    d_xin = nc.dram_tensor("xin", [66, et * BE], BF16, kind="ExternalInput")
    d_wih = nc.dram_tensor("wih_aug", [66, G], BF16, kind="ExternalInput")
    d_whh = nc.dram_tensor("whh_t", [H, G], BF16, kind="ExternalInput")
    d_bhhn = nc.dram_tensor("bhhn_row", [1, H], BF16, kind="ExternalInput")

    d_em1 = nc.dram_tensor("em_w1t", [2 * H, M1], BF16, kind="ExternalInput")
    d_em2 = nc.dram_tensor("em_w2t", [M1, M2], BF16, kind="ExternalInput")
    d_eow = nc.dram_tensor("eo_wt", [M2, H], BF16, kind="ExternalInput")
    d_dcw = nc.dram_tensor("dc_wt", [2 * H, G], BF16, kind="ExternalInput")
    d_midb = nc.dram_tensor("mid_bias", [1, 3584], BF16,
                            kind="ExternalInput")

    d_dwy = nc.dram_tensor("dwy_t", [NY, G], BF16, kind="ExternalInput")
    d_dwhh = nc.dram_tensor("dwhh_t", [H, G], BF16, kind="ExternalInput")
    d_dbhhn = nc.dram_tensor("dbhhn_row", [1, H], BF16, kind="ExternalInput")
    d_dm1 = nc.dram_tensor("dm_w1t", [H, M1], BF16, kind="ExternalInput")
    d_dm1b = nc.dram_tensor("dm_b1row", [1, M1], BF16, kind="ExternalInput")
    d_dm2 = nc.dram_tensor("dm_w2t", [M1, M2], BF16, kind="ExternalInput")
    d_dm2b = nc.dram_tensor("dm_b2row", [1, M2], BF16, kind="ExternalInput")
    d_dow = nc.dram_tensor("do_wt", [M2, NY], BF16, kind="ExternalInput")
    d_dobr = nc.dram_tensor("do_brow", [1, NY], BF16, kind="ExternalInput")
    d_xlast = nc.dram_tensor("xlast_t", [NX, BD], BF16, kind="ExternalInput")

    d_out = nc.dram_tensor("out", [hor * NY, BD], F32, kind="ExternalOutput")

    cc_in = nc.dram_tensor("cc_in", [BE, H], BF16)
    cc_out = nc.dram_tensor("cc_out", [NCORE, 16, H], BF16)

    with tile.TileContext(nc) as tc:
        with tc.tile_pool(name="pe", bufs=1) as pe, \
             tc.tile_pool(name="wts", bufs=1) as wts, \
             tc.tile_pool(name="xsp", bufs=2) as xsp, \
             tc.tile_pool(name="st", bufs=2) as st, \
             tc.tile_pool(name="tp", bufs=2) as tp, \
             tc.tile_pool(name="md", bufs=1) as md, \
             tc.tile_pool(name="pA", bufs=2, space="PSUM") as pA, \
             tc.tile_pool(name="pB", bufs=2, space="PSUM") as pB, \
             tc.tile_pool(name="pC", bufs=2, space="PSUM") as pC, \
             tc.tile_pool(name="pD", bufs=1, space="PSUM") as pD, \
             tc.tile_pool(name="pTR", bufs=1, space="PSUM") as pTR:

            # ---------- constants ----------
            idf = pe.tile([128, 128], F32, tag="idf")
            make_identity(nc, idf[:])
            idb = pe.tile([128, 128], BF16, tag="idb")
            nc.gpsimd.tensor_copy(idb[:], idf[:])
            ones_b = pe.tile([1, 128], BF16, tag="ones_b")
            nc.gpsimd.memset(ones_b[:], 1.0)
            zero_b = pe.tile([128, 512], BF16, tag="zero_b")
            nc.gpsimd.memset(zero_b[:], 0.0)

            def load_direct(pool, dram_ap, rows, cols, tag):
                r = pool.tile([rows, cols], BF16, tag=tag)
                nc.sync.dma_start(r[:], dram_ap)
                return r

            # Middle/decoder weights: allocate now, DMA lazily inside the
            # encode loop (one tile per step) so the startup xin load isn't
            # queued behind ~6MB of weight traffic.
            wload = []

            def load_lazy(dram_ap, rows, cols, tag, rdt=BF16):
                r = wts.tile([rows, cols], rdt, tag=tag)
                wload.append((r, dram_ap))
                return r

            # ---------- encoder weights (needed immediately) ----------
            wih_b = load_direct(wts, d_wih[:], 66, G, "wih")
            whh_b = [load_direct(wts, d_whh[128 * c:128 * (c + 1), :],
                                 128, G, f"whh{c}") for c in range(4)]
            ebhhn = load_direct(wts, d_bhhn[:], 1, H, "ebhhn")

            # ---------- encoder state ----------
            hT = pe.tile([128, H], BF16, tag="hT0")       # [feat%128, 4x128b]
            nc.vector.tensor_copy(hT[:], zero_b[:])
            h_bh = pe.tile([BE, H], BF16, tag="h0")       # [batch, feat]
            nc.gpsimd.memset(h_bh[:], 0.0)

            # ---------- middle + decoder weights (lazy bf16 DMA) ----------
            em1_b = [load_lazy(d_em1[128 * c:128 * (c + 1), :],
                               128, M1, f"em1_{c}") for c in range(8)]
            em2_b = [load_lazy(d_em2[128 * c:128 * (c + 1), :],
                               128, M2, f"em2_{c}") for c in range(8)]
            eo_b = [load_lazy(d_eow[128 * c:128 * (c + 1), :],
                              128, H, f"eo_{c}") for c in range(4)]
            dcw_b = [load_lazy(d_dcw[128 * c:128 * (c + 1), :],
                               128, G, f"dcw_{c}") for c in range(8)]
            bias_b = pe.tile([1, 3584], BF16, tag="bias_b")
            wload.append((bias_b, d_midb[:]))

            dwhh_b = [load_lazy(d_dwhh[128 * c:128 * (c + 1), :],
                                128, G, f"dwhh{c}") for c in range(4)]
            dm1_b = [load_lazy(d_dm1[128 * c:128 * (c + 1), :],
                               128, M1, f"dm1_{c}") for c in range(4)]
            dm2_b = [load_lazy(d_dm2[128 * c:128 * (c + 1), :],
                               128, M2, f"dm2_{c}") for c in range(8)]
            dow_b = [load_lazy(d_dow[128 * c:128 * (c + 1), :],
                               128, NY, f"dow_{c}") for c in range(4)]
            dbhhn_s = load_lazy(d_dbhhn[:], 1, H, "dbhhn")
            dm1b_s = load_lazy(d_dm1b[:], 1, M1, "dm1b")
            dm2b_s = load_lazy(d_dm2b[:], 1, M2, "dm2b")
            dob_r = load_direct(wts, d_dobr[:], 1, NY, "dobr")
            # ycw: rows 0:64 = Wy^T, rows 64:96 = const (filled post-middle).
            ycw = pe.tile([96, G], BF16, tag="ycw")
            wload.append((ycw[0:NY, :], d_dwy[:]))
            # ypc: rows 0:64 = y_t, rows 64:96 = I32 (selects const rows).
            ypc = pe.tile([96, BD], BF16, tag="ypc")
            nc.sync.dma_start(ypc[0:NX, :], d_xlast[:])
            nc.gpsimd.tensor_copy(ypc[64:96, :], idb[0:32, 0:32])

            # =======================================================
            # Encode loop, software-pipelined in feature halves.
            # =======================================================
            def enc_alloc():
                ga = pA.tile([BE, 512], F32, tag="A")
                gb = pB.tile([BE, 512], F32, tag="B")
                gc = pC.tile([BE, 512], F32, tag="C")
                gd = pD.tile([BE, 512], F32, tag="D")
                return ga, gb, gc, gd

            def enc_xs_mms(xs, ga, gb, gc):
                nc.tensor.matmul(ga[:], xs[:], wih_b[:, 0:512],
                                 start=True, stop=False)
                nc.tensor.matmul(gb[:], xs[:], wih_b[:, 512:1024],
                                 start=True, stop=False)
                nc.tensor.matmul(gc[:], xs[:], wih_b[:, 1024:1536],
                                 start=True, stop=True)

            def enc_bias_mm(gd):
                nc.tensor.matmul(gd[:], ones_b[0:1, 0:BE], ebhhn[:],
                                 start=True, stop=False)

            def load_xs(t):
                xb = xsp.tile([66, 128], BF16, tag="xs_b")
                nc.sync.dma_start(xb[:], d_xin[:, t * BE:(t + 1) * BE])
                return xb

            xs = load_xs(0)
            ga, gb, gc, gd = enc_alloc()
            enc_xs_mms(xs, ga, gb, gc)
            enc_bias_mm(gd)

            for t in range(et):
                last = (t == et - 1)
                # h-side matmuls, bank-major: r-gates, z-gates, n-h-gates
                # (z early so b=z*h and omz are off the critical path).
                for c in range(4):
                    nc.tensor.matmul(ga[:], hT[:, 128 * c:128 * (c + 1)],
                                     whh_b[c][:, 0:512],
                                     start=False, stop=(c == 3))
                for c in range(4):
                    nc.tensor.matmul(gb[:], hT[:, 128 * c:128 * (c + 1)],
                                     whh_b[c][:, 512:1024],
                                     start=False, stop=(c == 3))
                for c in range(4):
                    nc.tensor.matmul(gd[:], hT[:, 128 * c:128 * (c + 1)],
                                     whh_b[c][:, 1024:1536],
                                     start=False, stop=(c == 3))
                if not last:
                    xs_n = load_xs(t + 1)
                    if t < len(wload):
                        wa, wd = wload[t]
                        nc.sync.dma_start(wa[:], wd)
                    ga_n, gb_n, gc_n, gd_n = enc_alloc()

                # ---- eltwise: h' = (1-z)*n + z*h; rhn/npre and the tail
                # run in feature halves so tanh/hT chunks land early; dummy
                # transposes chained on eltwise temps keep the PE active
                # through the tail (HAM stays at K=8/8).
                r_t = tp.tile([BE, 512], BF16, tag="r")
                z_t = tp.tile([BE, 512], BF16, tag="z")
                n_t = tp.tile([BE, 512], BF16, tag="n")
                rhn = tp.tile([BE, 512], BF16, tag="rhn")
                npre = tp.tile([BE, 512], BF16, tag="npre")
                omz = tp.tile([BE, 512], BF16, tag="omz")
                b_t = tp.tile([BE, 512], BF16, tag="b")
                a_t = tp.tile([BE, 512], BF16, tag="a")
                h_new = st.tile([BE, H], BF16, tag="h")
                ptr = pTR.tile([128, 512], BF16, tag="TR")
                hT_new = st.tile([128, H], BF16, tag="hT")

                sl = [slice(0, 256), slice(256, 512)]
                nc.scalar.activation(r_t[:], ga[:], AF.Sigmoid)
                nc.scalar.activation(z_t[:], gb[:], AF.Sigmoid)
                nc.vector.tensor_scalar(omz[:], z_t[:], -1.0, 1.0,
                                        OP.mult, OP.add)
                nc.gpsimd.tensor_mul(b_t[:, sl[0]], z_t[:, sl[0]],
                                     h_bh[:, sl[0]])
                nc.gpsimd.tensor_mul(b_t[:, sl[1]], z_t[:, sl[1]],
                                     h_bh[:, sl[1]])
                for s in range(2):
                    nc.vector.tensor_mul(rhn[:, sl[s]], r_t[:, sl[s]],
                                         gd[:, sl[s]])
                    nc.vector.tensor_add(npre[:, sl[s]], rhn[:, sl[s]],
                                         gc[:, sl[s]])
                    nc.scalar.activation(n_t[:, sl[s]], npre[:, sl[s]],
                                         AF.Tanh)
                if not last:
                    # Interleave the (data-independent) next-step xs matmuls
                    # with filler transposes gated on eltwise temps so the
                    # PE keeps a high duty cycle through the tail and HAM
                    # stays at K=8/8 into the next gate burst.
                    nc.tensor.transpose(ptr[:, 0:128], r_t[:, 0:128],
                                        idb[:])
                    nc.tensor.transpose(ptr[:, 128:256], r_t[:, 128:256],
                                        idb[:])
                    nc.tensor.matmul(ga_n[:], xs_n[:], wih_b[:, 0:512],
                                     start=True, stop=False)
                    nc.tensor.transpose(ptr[:, 256:384], npre[:, 0:128],
                                        idb[:])
                    nc.tensor.transpose(ptr[:, 384:512], npre[:, 128:256],
                                        idb[:])
                    nc.tensor.matmul(gb_n[:], xs_n[:], wih_b[:, 512:1024],
                                     start=True, stop=False)
                    nc.tensor.transpose(ptr[:, 0:128], n_t[:, 0:128],
                                        idb[:])
                    nc.tensor.transpose(ptr[:, 128:256], n_t[:, 128:256],
                                        idb[:])
                    nc.tensor.matmul(gc_n[:], xs_n[:], wih_b[:, 1024:1536],
                                     start=True, stop=True)
                for s in range(2):
                    nc.vector.tensor_mul(a_t[:, sl[s]], omz[:, sl[s]],
                                         n_t[:, sl[s]])
                    nc.vector.tensor_add(h_new[:, sl[s]], a_t[:, sl[s]],
                                         b_t[:, sl[s]])
                    if not last:
                        for c in (2 * s, 2 * s + 1):
                            nc.tensor.transpose(
                                ptr[:, 128 * c:128 * (c + 1)],
                                h_new[:, 128 * c:128 * (c + 1)], idb[:])
                        if s == 0:
                            nc.scalar.copy(hT_new[:, sl[0]], ptr[:, sl[0]])
                        else:
                            nc.vector.tensor_copy(hT_new[:, sl[1]],
                                                  ptr[:, sl[1]])
                if not last:
                    enc_bias_mm(gd_n)
                    hT = hT_new
                    ga, gb, gc, gd = ga_n, gb_n, gc_n, gd_n
                h_bh = h_new

            # ---------- reshard: AllToAll of 16-row slices (bf16) ----------
            nc.sync.dma_start(cc_in[:], h_bh[:])
            nc.gpsimd.collective_compute(
                "AllToAll", OP.bypass,
                replica_groups=[list(range(NCORE))],
                ins=[cc_in[:]], outs=[cc_out[:]])

            pxa = md.tile([BD, H], BF16, tag="pA")
            pxb = md.tile([BD, H], BF16, tag="pB")
            pya = md.tile([BD, H], BF16, tag="pA")
            pyb = md.tile([BD, H], BF16, tag="pB")
            nc.sync.dma_start(pxa[0:16, :], cc_out[0][:])
            nc.sync.dma_start(pxa[16:32, :], cc_out[1][:])
            nc.sync.dma_start(pxb[0:16, :], cc_out[2][:])
            nc.sync.dma_start(pxb[16:32, :], cc_out[3][:])
            nc.sync.dma_start(pya[0:16, :], cc_out[4][:])
            nc.sync.dma_start(pya[16:32, :], cc_out[5][:])
            nc.sync.dma_start(pyb[0:16, :], cc_out[6][:])
            nc.sync.dma_start(pyb[16:32, :], cc_out[7][:])
            # PE warmup off the collective outputs: a dense burst of
            # transposes re-engages HAM before the middle MLP runs.
            pwu = pTR.tile([128, 512], BF16, tag="TR")
            for i, src in enumerate((pxa, pxb, pya, pyb)):
                for c in range(4):
                    nc.tensor.transpose(
                        pwu[:, (4 * i + c) * 32:(4 * i + c + 1) * 32],
                        src[:, 128 * c:128 * (c + 1)], idb[0:32, 0:32])
            hx = md.tile([BD, H], F32, tag="hx")
            hy = md.tile([BD, H], F32, tag="hy")
            nc.vector.tensor_add(hx[:], pxa[:], pxb[:])
            nc.vector.tensor_add(hy[:], pya[:], pyb[:])

            def trsp_b(src, cols, tag):
                """src [BD, cols] f32 -> bf16 [128, (cols//128)*BD] via PE."""
                nch = cols // 128
                p = pTR.tile([128, 512], F32, tag="TR")
                for c in range(nch):
                    nc.tensor.transpose(p[:, BD * c:BD * (c + 1)],
                                        src[:, 128 * c:128 * (c + 1)],
                                        idf[0:32, 0:32])
                o = md.tile([128, nch * BD], BF16, tag=tag)
                nc.scalar.copy(o[:], p[:, 0:nch * BD])
                return o

            hxT = trsp_b(hx, H, "hxT")
            hyT = trsp_b(hy, H, "hyT")

            # ---- middle MLP (batch-major, activations stationary) ----
            m1a = pA.tile([BD, 512], F32, tag="A")
            m1b = pB.tile([BD, 512], F32, tag="B")
            for c in range(8):
                wt = em1_b[c]
                s = (hxT if c < 4 else hyT)[:, BD * (c % 4):BD * (c % 4 + 1)]
                nc.tensor.matmul(m1a[:], s, wt[:, 0:512],
                                 start=(c == 0), stop=False)
                nc.tensor.matmul(m1b[:], s, wt[:, 512:1024],
                                 start=(c == 0), stop=False)
            nc.tensor.matmul(m1a[:], ones_b[0:1, 0:BD],
                             bias_b[0:1, 1536:2048], start=False, stop=True)
            nc.tensor.matmul(m1b[:], ones_b[0:1, 0:BD],
                             bias_b[0:1, 2048:2560], start=False, stop=True)
            hm1 = md.tile([BD, M1], F32, tag="hm1")
            nc.scalar.activation(hm1[:, 0:512], m1a[:], AF.Relu)
            nc.scalar.activation(hm1[:, 512:1024], m1b[:], AF.Relu)
            hm1T = trsp_b(hm1, M1, "hm1T_m")

            m2 = pC.tile([BD, M2], F32, tag="C")
            for c in range(8):
                nc.tensor.matmul(m2[:], hm1T[:, BD * c:BD * (c + 1)],
                                 em2_b[c][:], start=(c == 0), stop=False)
            nc.tensor.matmul(m2[:], ones_b[0:1, 0:BD], bias_b[0:1, 2560:3072],
                             start=False, stop=True)
            hm2 = md.tile([BD, M2], F32, tag="hm2")
            nc.scalar.activation(hm2[:], m2[:], AF.Relu)
            hm2T = trsp_b(hm2, M2, "hm2T_m")

            zp = pD.tile([BD, 512], F32, tag="D")
            for c in range(4):
                nc.tensor.matmul(zp[:, 0:H], hm2T[:, BD * c:BD * (c + 1)],
                                 eo_b[c][:], start=(c == 0), stop=False)
            nc.tensor.matmul(zp[:, 0:H], ones_b[0:1, 0:BD],
                             bias_b[0:1, 3072:3584], start=False, stop=True)
            z_sb = md.tile([BD, H], F32, tag="z_sb")
            nc.scalar.copy(z_sb[:], zp[:, 0:H])
            zT = trsp_b(z_sb, H, "zT")

            # const = cat(h_x, z) @ d_Wih[:, :2H].T + d_bih + d_bhh(r,z)
            cpa = pA.tile([BD, 512], F32, tag="A")
            cpb = pB.tile([BD, 512], F32, tag="B")
            cpn = pC.tile([BD, 512], F32, tag="C")
            for c in range(8):
                wt = dcw_b[c]
                s = (hxT if c < 4 else zT)[:, BD * (c % 4):BD * (c % 4 + 1)]
                nc.tensor.matmul(cpa[:], s, wt[:, 0:512],
                                 start=(c == 0), stop=False)
                nc.tensor.matmul(cpb[:], s, wt[:, 512:1024],
                                 start=(c == 0), stop=False)
                nc.tensor.matmul(cpn[:], s, wt[:, 1024:1536],
                                 start=(c == 0), stop=False)
            nc.tensor.matmul(cpa[:], ones_b[0:1, 0:BD],
                             bias_b[0:1, 0:512], start=False, stop=True)
            nc.tensor.matmul(cpb[:], ones_b[0:1, 0:BD],
                             bias_b[0:1, 512:1024], start=False, stop=True)
            nc.tensor.matmul(cpn[:], ones_b[0:1, 0:BD],
                             bias_b[0:1, 1024:1536], start=False, stop=True)
            nc.vector.tensor_copy(ycw[64:96, 0:512], cpa[:])
            nc.vector.tensor_copy(ycw[64:96, 512:1024], cpb[:])
            nc.vector.tensor_copy(ycw[64:96, 1024:1536], cpn[:])

            # =======================================================
            # Decode loop: fully transposed, h as [128, 4*32] bf16.
            # =======================================================
            hbf = st.tile([128, 4 * BD], BF16, tag="hbf")
            nc.gpsimd.memset(hbf[:], 0.0)

            def dec_whh(rz, hgn, hbf_src):
                for j in range(8):
                    for k in range(4):
                        nc.tensor.matmul(
                            rz[:, BD * j:BD * (j + 1)],
                            dwhh_b[k][:, 128 * j:128 * (j + 1)],
                            hbf_src[:, BD * k:BD * (k + 1)],
                            start=(j == 0 and k == 0), stop=False)
                for j in range(4):
                    for k in range(4):
                        nc.tensor.matmul(
                            hgn[:, BD * j:BD * (j + 1)],
                            dwhh_b[k][:, 128 * (8 + j):128 * (9 + j)],
                            hbf_src[:, BD * k:BD * (k + 1)],
                            start=(j == 0 and k == 0), stop=False)
                    nc.tensor.matmul(hgn[:, BD * j:BD * (j + 1)],
                                     dbhhn_s[0:1, 128 * j:128 * (j + 1)],
                                     ones_b[0:1, 0:BD],
                                     start=False, stop=(j == 3))

            def dec_m1bias(m1):
                for j in range(8):
                    nc.tensor.matmul(m1[:, BD * j:BD * (j + 1)],
                                     dm1b_s[0:1, 128 * j:128 * (j + 1)],
                                     ones_b[0:1, 0:BD],
                                     start=(j == 0), stop=False)

            def dec_m2bias(m2d):
                for j in range(4):
                    nc.tensor.matmul(m2d[:, BD * j:BD * (j + 1)],
                                     dm2b_s[0:1, 128 * j:128 * (j + 1)],
                                     ones_b[0:1, 0:BD],
                                     start=(j == 0), stop=False)

            rz = pA.tile([128, 512], F32, tag="A")
            hgn = pB.tile([128, 512], F32, tag="B")
            m1 = pD.tile([128, 512], F32, tag="D")
            m2d = pTR.tile([128, 512], F32, tag="TR")
            dec_m1bias(m1)
            dec_m2bias(m2d)
            # t=0: h=0, so no Whh matmuls; hgn(0) = bias only.
            for j in range(4):
                nc.tensor.matmul(hgn[:, BD * j:BD * (j + 1)],
                                 dbhhn_s[0:1, 128 * j:128 * (j + 1)],
                                 ones_b[0:1, 0:BD],
                                 start=(j == 0), stop=(j == 3))

            for t in range(hor):
                lastd = (t == hor - 1)
                # ---- y/const-side gate matmuls ----
                an = pC.tile([128, 512], F32, tag="C")
                for j in range(8):
                    nc.tensor.matmul(rz[:, BD * j:BD * (j + 1)],
                                     ycw[:, 128 * j:128 * (j + 1)], ypc[:],
                                     start=(t == 0 and j == 0),
                                     stop=(j == 7))
                for j in range(4):
                    nc.tensor.matmul(an[:, BD * j:BD * (j + 1)],
                                     ycw[:, 128 * (8 + j):128 * (9 + j)],
                                     ypc[:], start=(j == 0), stop=(j == 3))

                # ---- GRU eltwise, transposed layout [128, 4*32]; tail in
                # chunk-pair halves so M1 starts on half 0 early ----
                r_t = tp.tile([128, 4 * BD], BF16, tag="dr")
                nc.scalar.activation(r_t[:], rz[:, 0:4 * BD], AF.Sigmoid)
                z_t = tp.tile([128, 4 * BD], BF16, tag="dz")
                nc.scalar.activation(z_t[:], rz[:, 4 * BD:8 * BD], AF.Sigmoid)
                omz = tp.tile([128, 4 * BD], BF16, tag="domz")
                nc.vector.tensor_scalar(omz[:], z_t[:], -1.0, 1.0,
                                        OP.mult, OP.add)
                u_t = tp.tile([128, 4 * BD], BF16, tag="du")
                nc.gpsimd.tensor_mul(u_t[:], z_t[:], hbf[:])
                t2 = tp.tile([128, 4 * BD], BF16, tag="dt2")
                npre = tp.tile([128, 4 * BD], BF16, tag="dnp")
                n_t = tp.tile([128, 4 * BD], BF16, tag="dn")
                a_t = tp.tile([128, 4 * BD], BF16, tag="da")
                hbf_n = st.tile([128, 4 * BD], BF16, tag="hbf")
                for s in range(2):
                    dsl = slice(2 * BD * s, 2 * BD * (s + 1))
                    nc.vector.tensor_mul(t2[:, dsl], r_t[:, dsl],
                                         hgn[:, dsl])
                    nc.vector.tensor_add(npre[:, dsl], t2[:, dsl],
                                         an[:, dsl])
                    nc.scalar.activation(n_t[:, dsl], npre[:, dsl], AF.Tanh)
                    nc.vector.tensor_mul(a_t[:, dsl], omz[:, dsl],
                                         n_t[:, dsl])
                    nc.vector.tensor_add(hbf_n[:, dsl], a_t[:, dsl],
                                         u_t[:, dsl])
                    # M1 on the two ready h chunks
                    for k in (2 * s, 2 * s + 1):
                        for j in range(8):
                            nc.tensor.matmul(
                                m1[:, BD * j:BD * (j + 1)],
                                dm1_b[k][:, 128 * j:128 * (j + 1)],
                                hbf_n[:, BD * k:BD * (k + 1)],
                                start=False, stop=(k == 3 and j == 7))
                hbf = hbf_n
                hm1_s = tp.tile([128, 8 * BD], BF16, tag="dhm1")
                nc.vector.tensor_scalar_max(hm1_s[:], m1[:, 0:8 * BD], 0.0)

                # ---- M2 ----
                for k in range(8):
                    for j in range(4):
                        nc.tensor.matmul(m2d[:, BD * j:BD * (j + 1)],
                                         dm2_b[k][:, 128 * j:128 * (j + 1)],
                                         hm1_s[:, BD * k:BD * (k + 1)],
                                         start=False,
                                         stop=(k == 7 and j == 3))
                hm2_s = tp.tile([128, 4 * BD], BF16, tag="dhm2")
                nc.vector.tensor_scalar_max(hm2_s[:], m2d[:, 0:4 * BD], 0.0)

                # bias pre-issue for t+1 (fills the out-matmul wait)
                if not lastd:
                    m1_n = pD.tile([128, 512], F32, tag="D")
                    dec_m1bias(m1_n)
                    m2_n = pTR.tile([128, 512], F32, tag="TR")
                    dec_m2bias(m2_n)

                # ---- output head: y [64, 32]; do_b rides as a K=1 matmul --
                yb = pC.tile([128, 512], F32, tag="C")
                nc.tensor.matmul(yb[0:NY, 0:BD], dob_r[0:1, 0:NY],
                                 ones_b[0:1, 0:BD], start=True, stop=False)
                for k in range(4):
                    nc.tensor.matmul(yb[0:NY, 0:BD],
                                     dow_b[k][:, 0:NY],
                                     hm2_s[:, BD * k:BD * (k + 1)],
                                     start=False, stop=(k == 3))
                if not lastd:
                    # critical path: feed y back (bf16) before the f32 copy
                    nc.vector.tensor_copy(ypc[0:NY, :], yb[0:NY, 0:BD])
                y_f = tp.tile([NY, BD], F32, tag="dy")
                nc.scalar.copy(y_f[:], yb[0:NY, 0:BD])
                nc.sync.dma_start(d_out[NY * t:NY * (t + 1), :], y_f[:])
                if not lastd:
                    # pre-issue next step's h-side matmuls
                    rz_n = pA.tile([128, 512], F32, tag="A")
                    hgn_n = pB.tile([128, 512], F32, tag="B")
                    dec_whh(rz_n, hgn_n, hbf)
                    rz, hgn, m1, m2d = rz_n, hgn_n, m1_n, m2_n

    nc.compile()
    return nc


# ---------------------------------------------------------------------------
# Host-side sharding
# ---------------------------------------------------------------------------

def shard_inputs(inp, et=100, hor=60):
    f32 = np.float32

    def bf(a):
        return np.ascontiguousarray(np.asarray(a, f32).astype(BF))

    x, y = np.asarray(inp["x"], f32), np.asarray(inp["y"], f32)
    chains = [("xf", False, x), ("xb", True, x),
              ("ef", False, y), ("eb", True, y)]
    in_maps = []
    shared = {}

    def wih_aug(pre):
        wih = np.asarray(inp[pre + "_Wih"], f32)
        bih = np.asarray(inp[pre + "_bih"], f32)
        bhh = np.asarray(inp[pre + "_bhh"], f32)
        aug = np.zeros((66, G), f32)
        aug[0:64, :] = wih.T
        bias = bih.copy()
        bias[0:2 * H] += bhh[0:2 * H]
        aug[64, :] = bias
        aug[65, H:2 * H] = BIG
        return bf(aug)

    d_Wih = np.asarray(inp["d_Wih"], f32)
    d_bih = np.asarray(inp["d_bih"], f32)
    d_bhh = np.asarray(inp["d_bhh"], f32)
    dc_b = d_bih.copy()
    dc_b[0:2 * H] += d_bhh[0:2 * H]

    shared["em_w1t"] = bf(np.asarray(inp["em_W1"], f32).T)
    shared["em_w2t"] = bf(np.asarray(inp["em_W2"], f32).T)
    shared["eo_wt"] = bf(np.asarray(inp["eo_W"], f32).T)
    shared["dc_wt"] = bf(d_Wih[:, 0:2 * H].T)
    midb = np.concatenate([dc_b, np.asarray(inp["em_b1"], f32),
                           np.asarray(inp["em_b2"], f32),
                           np.asarray(inp["eo_b"], f32)])[None, :]
    shared["mid_bias"] = bf(midb)
    shared["dwy_t"] = bf(d_Wih[:, 2 * H:].T)
    shared["dwhh_t"] = bf(np.asarray(inp["d_Whh"], f32).T)
    shared["dbhhn_row"] = bf(d_bhh[None, 2 * H:])
    shared["dm_w1t"] = bf(np.asarray(inp["dm_W1"], f32).T)
    shared["dm_b1row"] = bf(np.asarray(inp["dm_b1"], f32)[None, :])
    shared["dm_w2t"] = bf(np.asarray(inp["dm_W2"], f32).T)
    shared["dm_b2row"] = bf(np.asarray(inp["dm_b2"], f32)[None, :])
    shared["do_wt"] = bf(np.asarray(inp["do_W"], f32).T)
    shared["do_brow"] = bf(np.asarray(inp["do_b"], f32)[None, :])

    for j in range(NCORE):
        chain, half = j // 2, j % 2
        pre, rev, seq = chains[chain]
        T = seq.shape[1]
        s = seq[128 * half:128 * (half + 1)]          # [128, T, 64]
        xin = np.zeros((66, et, BE), f32)
        xin[64, :, :] = 1.0
        pad = et - T
        if pad:
            xin[65, 0:pad, :] = 1.0
        order = np.arange(T)[::-1] if rev else np.arange(T)
        xin[0:64, pad:, :] = s[:, order, :].transpose(2, 1, 0)
        m = dict(shared)
        m["xin"] = bf(xin.reshape(66, et * BE))
        m["wih_aug"] = wih_aug(pre)
        m["whh_t"] = bf(np.asarray(inp[pre + "_Whh"], f32).T)
        m["bhhn_row"] = bf(np.asarray(inp[pre + "_bhh"], f32)[None, 2 * H:])
        xl = np.concatenate([x[16 * j:16 * j + 16, -1, :],
                             x[128 + 16 * j:128 + 16 * j + 16, -1, :]])
        m["xlast_t"] = bf(xl.T)
        in_maps.append(m)
    return in_maps


def unshard(results, hor=60):
    out = np.zeros((B, hor, NY), np.float32)
    for j in range(NCORE):
        o = results[j]["out"].reshape(hor, NY, BD).transpose(2, 0, 1)
        out[16 * j:16 * j + 16] = o[0:16]
        out[128 + 16 * j:128 + 16 * j + 16] = o[16:32]
    return out


_NC = None


def kernel(**inputs):
    global _NC
    from concourse.bass_utils import run_bass_kernel_spmd
    if _NC is None:
        _NC = build_nc()
    in_maps = shard_inputs(inputs)
    res = run_bass_kernel_spmd(_NC, in_maps, core_ids=list(range(NCORE)))
    return unshard(res.results)


# revision 47
# speedup vs baseline: 1.0221x; 1.0159x over previous
"""Trainium2 Bass kernel for the GRU autoencoder (v8).

Distribution (8 NeuronCores):
  Encode : chain-parallel x batch-parallel. Core j handles GRU chain j//2
           (xf, xb, ef, eb) on batch half j%2 (128 rows), uniform 100-step
           loop; the 50-step x-chains get 50 exact identity steps (z forced
           to 1 via a +BIG flag row). AllToAll reshards 16-row slices so each
           core decodes global rows [16j:16j+16] u [128+16j:+16].

v8 (from 2.55ms baseline to 1.24ms): everything bf16 end-to-end (weights
shipped as bf16 from the host — no device-side casting, lazy weight DMA
spread across encode steps), encoder eltwise tail reordered/halved with
filler transposes + pre-issued input-side matmuls to hold HAM at K=8/8,
decoder fully transposed (weights-stationary, [feature, batch] layout,
N=32 moving matmuls, zero per-step transposes, biases as K=1 stationary-row
matmuls, relus on the DVE).
"""

import sys

sys.path.insert(0, "/opt/trn_rl_repo")

import ml_dtypes
import numpy as np

import concourse.bass as bass
import concourse.mybir as mybir
import concourse.tile as tile
from concourse import bacc
from concourse.masks import make_identity

dt = mybir.dt
AF = mybir.ActivationFunctionType
OP = mybir.AluOpType

B, TX, TY, NX, NY, H, HOR = 256, 50, 100, 64, 64, 512, 60
M1, M2 = 1024, 512
G = 3 * H
NCORE = 8
BE = 128   # encoder batch rows per core
BD = 32    # decoder batch rows per core
BIG = 30000.0

F32, BF16 = dt.float32, dt.bfloat16
BF = ml_dtypes.bfloat16


def build_nc(et=100, hor=60):
    nc = bacc.Bacc("TRN2", target_bir_lowering=False, debug=False,
                   num_devices=NCORE)

    # ---- DRAM parameters (bf16 except the ACT bias column) ----
    d_xin = nc.dram_tensor("xin", [66, et * BE], BF16, kind="ExternalInput")
    d_wih = nc.dram_tensor("wih_aug", [66, G], BF16, kind="ExternalInput")
    d_whh = nc.dram_tensor("whh_t", [H, G], BF16, kind="ExternalInput")
    d_bhhn = nc.dram_tensor("bhhn_row", [1, H], BF16, kind="ExternalInput")

    d_em1 = nc.dram_tensor("em_w1t", [2 * H, M1], BF16, kind="ExternalInput")
    d_em2 = nc.dram_tensor("em_w2t", [M1, M2], BF16, kind="ExternalInput")
    d_eow = nc.dram_tensor("eo_wt", [M2, H], BF16, kind="ExternalInput")
    d_dcw = nc.dram_tensor("dc_wt", [2 * H, G], BF16, kind="ExternalInput")
    d_midb = nc.dram_tensor("mid_bias", [1, 3584], BF16,
                            kind="ExternalInput")

    d_dwy = nc.dram_tensor("dwy_t", [NY, G], BF16, kind="ExternalInput")
    d_dwhh = nc.dram_tensor("dwhh_t", [H, G], BF16, kind="ExternalInput")
    d_dbhhn = nc.dram_tensor("dbhhn_row", [1, H], BF16, kind="ExternalInput")
    d_dm1 = nc.dram_tensor("dm_w1t", [H, M1], BF16, kind="ExternalInput")
    d_dm1b = nc.dram_tensor("dm_b1row", [1, M1], BF16, kind="ExternalInput")
    d_dm2 = nc.dram_tensor("dm_w2t", [M1, M2], BF16, kind="ExternalInput")
    d_dm2b = nc.dram_tensor("dm_b2row", [1, M2], BF16, kind="ExternalInput")
    d_dow = nc.dram_tensor("do_wt", [M2, NY], BF16, kind="ExternalInput")
    d_dobr = nc.dram_tensor("do_brow", [1, NY], BF16, kind="ExternalInput")
    d_xlast = nc.dram_tensor("xlast_t", [NX, BD], BF16, kind="ExternalInput")

    d_out = nc.dram_tensor("out", [hor * NY, BD], F32, kind="ExternalOutput")

    cc_in = nc.dram_tensor("cc_in", [BE, H], BF16)
    cc_out = nc.dram_tensor("cc_out", [NCORE, 16, H], BF16)

    with tile.TileContext(nc) as tc:
        with tc.tile_pool(name="pe", bufs=1) as pe, \
             tc.tile_pool(name="wts", bufs=1) as wts, \
             tc.tile_pool(name="xsp", bufs=3) as xsp, \
             tc.tile_pool(name="st", bufs=2) as st, \
             tc.tile_pool(name="tp", bufs=2) as tp, \
             tc.tile_pool(name="md", bufs=1) as md, \
             tc.tile_pool(name="pA", bufs=2, space="PSUM") as pA, \
             tc.tile_pool(name="pB", bufs=2, space="PSUM") as pB, \
             tc.tile_pool(name="pC", bufs=2, space="PSUM") as pC, \
             tc.tile_pool(name="pD", bufs=1, space="PSUM") as pD, \
             tc.tile_pool(name="pTR", bufs=1, space="PSUM") as pTR:

            # ---------- constants ----------
            idf = pe.tile([128, 128], F32, tag="idf")
            make_identity(nc, idf[:])
            idb = pe.tile([128, 128], BF16, tag="idb")
            nc.gpsimd.tensor_copy(idb[:], idf[:])
            ones_b = pe.tile([1, 128], BF16, tag="ones_b")
            nc.gpsimd.memset(ones_b[:], 1.0)
            zero_b = pe.tile([128, 512], BF16, tag="zero_b")
            nc.gpsimd.memset(zero_b[:], 0.0)

            def load_direct(pool, dram_ap, rows, cols, tag):
                r = pool.tile([rows, cols], BF16, tag=tag)
                nc.sync.dma_start(r[:], dram_ap)
                return r

            # Middle/decoder weights: allocate now, DMA lazily inside the
            # encode loop (one tile per step) so the startup xin load isn't
            # queued behind ~6MB of weight traffic.
            wload = []

            def load_lazy(dram_ap, rows, cols, tag, rdt=BF16):
                r = wts.tile([rows, cols], rdt, tag=tag)
                wload.append((r, dram_ap))
                return r

            # ---------- encoder weights (needed immediately) ----------
            wih_b = load_direct(wts, d_wih[:], 66, G, "wih")
            whh_b = [load_direct(wts, d_whh[128 * c:128 * (c + 1), :],
                                 128, G, f"whh{c}") for c in range(4)]
            ebhhn = load_direct(wts, d_bhhn[:], 1, H, "ebhhn")

            # ---------- encoder state ----------
            hT = pe.tile([128, H], BF16, tag="hT0")       # [feat%128, 4x128b]
            nc.vector.tensor_copy(hT[:], zero_b[:])
            h_bh = pe.tile([BE, H], BF16, tag="h0")       # [batch, feat]
            nc.gpsimd.memset(h_bh[:], 0.0)

            # ---------- middle + decoder weights (lazy bf16 DMA) ----------
            em1_b = [load_lazy(d_em1[128 * c:128 * (c + 1), :],
                               128, M1, f"em1_{c}") for c in range(8)]
            em2_b = [load_lazy(d_em2[128 * c:128 * (c + 1), :],
                               128, M2, f"em2_{c}") for c in range(8)]
            eo_b = [load_lazy(d_eow[128 * c:128 * (c + 1), :],
                              128, H, f"eo_{c}") for c in range(4)]
            dcw_b = [load_lazy(d_dcw[128 * c:128 * (c + 1), :],
                               128, G, f"dcw_{c}") for c in range(8)]
            bias_b = pe.tile([1, 3584], BF16, tag="bias_b")
            wload.append((bias_b, d_midb[:]))

            dwhh_b = [load_lazy(d_dwhh[128 * c:128 * (c + 1), :],
                                128, G, f"dwhh{c}") for c in range(4)]
            dm1_b = [load_lazy(d_dm1[128 * c:128 * (c + 1), :],
                               128, M1, f"dm1_{c}") for c in range(4)]
            dm2_b = [load_lazy(d_dm2[128 * c:128 * (c + 1), :],
                               128, M2, f"dm2_{c}") for c in range(8)]
            dow_b = [load_lazy(d_dow[128 * c:128 * (c + 1), :],
                               128, NY, f"dow_{c}") for c in range(4)]
            dbhhn_s = load_lazy(d_dbhhn[:], 1, H, "dbhhn")
            dm1b_s = load_lazy(d_dm1b[:], 1, M1, "dm1b")
            dm2b_s = load_lazy(d_dm2b[:], 1, M2, "dm2b")
            dob_r = load_direct(wts, d_dobr[:], 1, NY, "dobr")
            # ycw: rows 0:64 = Wy^T, rows 64:96 = const (filled post-middle).
            ycw = pe.tile([96, G], BF16, tag="ycw")
            wload.append((ycw[0:NY, :], d_dwy[:]))
            # ypc: rows 0:64 = y_t, rows 64:96 = I32 (selects const rows).
            ypc = pe.tile([96, BD], BF16, tag="ypc")
            nc.sync.dma_start(ypc[0:NX, :], d_xlast[:])
            nc.gpsimd.tensor_copy(ypc[64:96, :], idb[0:32, 0:32])

            # =======================================================
            # Encode loop, software-pipelined in feature halves.
            # =======================================================
            def enc_alloc():
                ga = pA.tile([BE, 512], F32, tag="A")
                gb = pB.tile([BE, 512], F32, tag="B")
                gc = pC.tile([BE, 512], F32, tag="C")
                gd = pD.tile([BE, 512], F32, tag="D")
                return ga, gb, gc, gd

            def enc_xs_mms(xs, ga, gb, gc):
                nc.tensor.matmul(ga[:], xs[:], wih_b[:, 0:512],
                                 start=True, stop=False)
                nc.tensor.matmul(gb[:], xs[:], wih_b[:, 512:1024],
                                 start=True, stop=False)
                nc.tensor.matmul(gc[:], xs[:], wih_b[:, 1024:1536],
                                 start=True, stop=True)

            def enc_bias_mm(gd):
                nc.tensor.matmul(gd[:], ones_b[0:1, 0:BE], ebhhn[:],
                                 start=True, stop=False)

            def load_xs(t):
                xb = xsp.tile([66, 128], BF16, tag="xs_b")
                nc.sync.dma_start(xb[:], d_xin[:, t * BE:(t + 1) * BE])
                return xb

            xs = load_xs(0)
            xs_n = load_xs(1)
            ga, gb, gc, gd = enc_alloc()
            enc_xs_mms(xs, ga, gb, gc)
            enc_bias_mm(gd)

            for t in range(et):
                last = (t == et - 1)
                # h-side matmuls, bank-major: r-gates, z-gates, n-h-gates
                # (z early so b=z*h and omz are off the critical path).
                for c in range(4):
                    nc.tensor.matmul(ga[:], hT[:, 128 * c:128 * (c + 1)],
                                     whh_b[c][:, 0:512],
                                     start=False, stop=(c == 3))
                for c in range(4):
                    nc.tensor.matmul(gb[:], hT[:, 128 * c:128 * (c + 1)],
                                     whh_b[c][:, 512:1024],
                                     start=False, stop=(c == 3))
                for c in range(4):
                    nc.tensor.matmul(gd[:], hT[:, 128 * c:128 * (c + 1)],
                                     whh_b[c][:, 1024:1536],
                                     start=False, stop=(c == 3))
                if not last:
                    # xin prefetched 2 steps ahead, issued BEFORE the lazy
                    # weight-tile DMA so it never queues behind ~5us of
                    # weight traffic (was stretching every ~3rd step).
                    if t + 2 < et:
                        xs_nn = load_xs(t + 2)
                    if t < len(wload):
                        wa, wd = wload[t]
                        nc.sync.dma_start(wa[:], wd)
                    ga_n, gb_n, gc_n, gd_n = enc_alloc()

                # ---- eltwise: h' = (1-z)*n + z*h; rhn/npre and the tail
                # run in feature halves so tanh/hT chunks land early; dummy
                # transposes chained on eltwise temps keep the PE active
                # through the tail (HAM stays at K=8/8).
                r_t = tp.tile([BE, 512], BF16, tag="r")
                z_t = tp.tile([BE, 512], BF16, tag="z")
                n_t = tp.tile([BE, 512], BF16, tag="n")
                rhn = tp.tile([BE, 512], BF16, tag="rhn")
                npre = tp.tile([BE, 512], BF16, tag="npre")
                omz = tp.tile([BE, 512], BF16, tag="omz")
                b_t = tp.tile([BE, 512], BF16, tag="b")
                a_t = tp.tile([BE, 512], BF16, tag="a")
                h_new = st.tile([BE, H], BF16, tag="h")
                ptr = pTR.tile([128, 512], BF16, tag="TR")
                hT_new = st.tile([128, H], BF16, tag="hT")

                sl = [slice(0, 256), slice(256, 512)]
                nc.scalar.activation(r_t[:], ga[:], AF.Sigmoid)
                nc.scalar.activation(z_t[:], gb[:], AF.Sigmoid)
                nc.vector.tensor_scalar(omz[:], z_t[:], -1.0, 1.0,
                                        OP.mult, OP.add)
                nc.gpsimd.tensor_mul(b_t[:, sl[0]], z_t[:, sl[0]],
                                     h_bh[:, sl[0]])
                nc.gpsimd.tensor_mul(b_t[:, sl[1]], z_t[:, sl[1]],
                                     h_bh[:, sl[1]])
                for s in range(2):
                    nc.vector.tensor_mul(rhn[:, sl[s]], r_t[:, sl[s]],
                                         gd[:, sl[s]])
                    nc.vector.tensor_add(npre[:, sl[s]], rhn[:, sl[s]],
                                         gc[:, sl[s]])
                    nc.scalar.activation(n_t[:, sl[s]], npre[:, sl[s]],
                                         AF.Tanh)
                if not last:
                    # Interleave the (data-independent) next-step xs matmuls
                    # with filler transposes gated on eltwise temps so the
                    # PE keeps a high duty cycle through the tail and HAM
                    # stays at K=8/8 into the next gate burst.
                    nc.tensor.transpose(ptr[:, 0:128], r_t[:, 0:128],
                                        idb[:])
                    nc.tensor.transpose(ptr[:, 128:256], r_t[:, 128:256],
                                        idb[:])
                    nc.tensor.matmul(ga_n[:], xs_n[:], wih_b[:, 0:512],
                                     start=True, stop=False)
                    nc.tensor.transpose(ptr[:, 256:384], npre[:, 0:128],
                                        idb[:])
                    nc.tensor.transpose(ptr[:, 384:512], npre[:, 128:256],
                                        idb[:])
                    nc.tensor.matmul(gb_n[:], xs_n[:], wih_b[:, 512:1024],
                                     start=True, stop=False)
                    nc.tensor.transpose(ptr[:, 0:128], n_t[:, 0:128],
                                        idb[:])
                    nc.tensor.transpose(ptr[:, 128:256], n_t[:, 128:256],
                                        idb[:])
                    nc.tensor.matmul(gc_n[:], xs_n[:], wih_b[:, 1024:1536],
                                     start=True, stop=True)
                for s in range(2):
                    nc.vector.tensor_mul(a_t[:, sl[s]], omz[:, sl[s]],
                                         n_t[:, sl[s]])
                    nc.vector.tensor_add(h_new[:, sl[s]], a_t[:, sl[s]],
                                         b_t[:, sl[s]])
                    if not last:
                        for c in (2 * s, 2 * s + 1):
                            nc.tensor.transpose(
                                ptr[:, 128 * c:128 * (c + 1)],
                                h_new[:, 128 * c:128 * (c + 1)], idb[:])
                        if s == 0:
                            nc.scalar.copy(hT_new[:, sl[0]], ptr[:, sl[0]])
                        else:
                            nc.vector.tensor_copy(hT_new[:, sl[1]],
                                                  ptr[:, sl[1]])
                if not last:
                    enc_bias_mm(gd_n)
                    hT = hT_new
                    ga, gb, gc, gd = ga_n, gb_n, gc_n, gd_n
                    xs, xs_n = xs_n, (xs_nn if t + 2 < et else None)
                h_bh = h_new

            # ---------- reshard: AllToAll of 16-row slices (bf16) ----------
            nc.sync.dma_start(cc_in[:], h_bh[:])
            nc.gpsimd.collective_compute(
                "AllToAll", OP.bypass,
                replica_groups=[list(range(NCORE))],
                ins=[cc_in[:]], outs=[cc_out[:]])

            pxa = md.tile([BD, H], BF16, tag="pA")
            pxb = md.tile([BD, H], BF16, tag="pB")
            pya = md.tile([BD, H], BF16, tag="pA")
            pyb = md.tile([BD, H], BF16, tag="pB")
            nc.sync.dma_start(pxa[0:16, :], cc_out[0][:])
            nc.sync.dma_start(pxa[16:32, :], cc_out[1][:])
            nc.sync.dma_start(pxb[0:16, :], cc_out[2][:])
            nc.sync.dma_start(pxb[16:32, :], cc_out[3][:])
            nc.sync.dma_start(pya[0:16, :], cc_out[4][:])
            nc.sync.dma_start(pya[16:32, :], cc_out[5][:])
            nc.sync.dma_start(pyb[0:16, :], cc_out[6][:])
            nc.sync.dma_start(pyb[16:32, :], cc_out[7][:])
            # PE warmup off the collective outputs: a dense burst of
            # transposes re-engages HAM before the middle MLP runs.
            pwu = pTR.tile([128, 512], BF16, tag="TR")
            for i, src in enumerate((pxa, pxb, pya, pyb)):
                for c in range(4):
                    nc.tensor.transpose(
                        pwu[:, (4 * i + c) * 32:(4 * i + c + 1) * 32],
                        src[:, 128 * c:128 * (c + 1)], idb[0:32, 0:32])
            hx = md.tile([BD, H], F32, tag="hx")
            hy = md.tile([BD, H], F32, tag="hy")
            nc.vector.tensor_add(hx[:], pxa[:], pxb[:])
            nc.vector.tensor_add(hy[:], pya[:], pyb[:])

            def trsp_b(src, cols, tag):
                """src [BD, cols] f32 -> bf16 [128, (cols//128)*BD] via PE."""
                nch = cols // 128
                p = pTR.tile([128, 512], F32, tag="TR")
                for c in range(nch):
                    nc.tensor.transpose(p[:, BD * c:BD * (c + 1)],
                                        src[:, 128 * c:128 * (c + 1)],
                                        idf[0:32, 0:32])
                o = md.tile([128, nch * BD], BF16, tag=tag)
                nc.scalar.copy(o[:], p[:, 0:nch * BD])
                return o

            hxT = trsp_b(hx, H, "hxT")
            hyT = trsp_b(hy, H, "hyT")

            # ---- middle MLP (batch-major, activations stationary) ----
            m1a = pA.tile([BD, 512], F32, tag="A")
            m1b = pB.tile([BD, 512], F32, tag="B")
            for c in range(8):
                wt = em1_b[c]
                s = (hxT if c < 4 else hyT)[:, BD * (c % 4):BD * (c % 4 + 1)]
                nc.tensor.matmul(m1a[:], s, wt[:, 0:512],
                                 start=(c == 0), stop=False)
                nc.tensor.matmul(m1b[:], s, wt[:, 512:1024],
                                 start=(c == 0), stop=False)
            nc.tensor.matmul(m1a[:], ones_b[0:1, 0:BD],
                             bias_b[0:1, 1536:2048], start=False, stop=True)
            nc.tensor.matmul(m1b[:], ones_b[0:1, 0:BD],
                             bias_b[0:1, 2048:2560], start=False, stop=True)
            hm1 = md.tile([BD, M1], F32, tag="hm1")
            nc.scalar.activation(hm1[:, 0:512], m1a[:], AF.Relu)
            nc.scalar.activation(hm1[:, 512:1024], m1b[:], AF.Relu)
            hm1T = trsp_b(hm1, M1, "hm1T_m")

            m2 = pC.tile([BD, M2], F32, tag="C")
            for c in range(8):
                nc.tensor.matmul(m2[:], hm1T[:, BD * c:BD * (c + 1)],
                                 em2_b[c][:], start=(c == 0), stop=False)
            nc.tensor.matmul(m2[:], ones_b[0:1, 0:BD], bias_b[0:1, 2560:3072],
                             start=False, stop=True)
            hm2 = md.tile([BD, M2], F32, tag="hm2")
            nc.scalar.activation(hm2[:], m2[:], AF.Relu)
            hm2T = trsp_b(hm2, M2, "hm2T_m")

            zp = pD.tile([BD, 512], F32, tag="D")
            for c in range(4):
                nc.tensor.matmul(zp[:, 0:H], hm2T[:, BD * c:BD * (c + 1)],
                                 eo_b[c][:], start=(c == 0), stop=False)
            nc.tensor.matmul(zp[:, 0:H], ones_b[0:1, 0:BD],
                             bias_b[0:1, 3072:3584], start=False, stop=True)
            z_sb = md.tile([BD, H], F32, tag="z_sb")
            nc.scalar.copy(z_sb[:], zp[:, 0:H])
            zT = trsp_b(z_sb, H, "zT")

            # const = cat(h_x, z) @ d_Wih[:, :2H].T + d_bih + d_bhh(r,z)
            cpa = pA.tile([BD, 512], F32, tag="A")
            cpb = pB.tile([BD, 512], F32, tag="B")
            cpn = pC.tile([BD, 512], F32, tag="C")
            for c in range(8):
                wt = dcw_b[c]
                s = (hxT if c < 4 else zT)[:, BD * (c % 4):BD * (c % 4 + 1)]
                nc.tensor.matmul(cpa[:], s, wt[:, 0:512],
                                 start=(c == 0), stop=False)
                nc.tensor.matmul(cpb[:], s, wt[:, 512:1024],
                                 start=(c == 0), stop=False)
                nc.tensor.matmul(cpn[:], s, wt[:, 1024:1536],
                                 start=(c == 0), stop=False)
            nc.tensor.matmul(cpa[:], ones_b[0:1, 0:BD],
                             bias_b[0:1, 0:512], start=False, stop=True)
            nc.tensor.matmul(cpb[:], ones_b[0:1, 0:BD],
                             bias_b[0:1, 512:1024], start=False, stop=True)
            nc.tensor.matmul(cpn[:], ones_b[0:1, 0:BD],
                             bias_b[0:1, 1024:1536], start=False, stop=True)
            nc.vector.tensor_copy(ycw[64:96, 0:512], cpa[:])
            nc.vector.tensor_copy(ycw[64:96, 512:1024], cpb[:])
            nc.vector.tensor_copy(ycw[64:96, 1024:1536], cpn[:])

            # =======================================================
            # Decode loop: fully transposed, h as [128, 4*32] bf16.
            # =======================================================
            hbf = st.tile([128, 4 * BD], BF16, tag="hbf")
            nc.gpsimd.memset(hbf[:], 0.0)

            def dec_whh(rz, hgn, hbf_src):
                for j in range(8):
                    for k in range(4):
                        nc.tensor.matmul(
                            rz[:, BD * j:BD * (j + 1)],
                            dwhh_b[k][:, 128 * j:128 * (j + 1)],
                            hbf_src[:, BD * k:BD * (k + 1)],
                            start=(j == 0 and k == 0), stop=False)
                for j in range(4):
                    for k in range(4):
                        nc.tensor.matmul(
                            hgn[:, BD * j:BD * (j + 1)],
                            dwhh_b[k][:, 128 * (8 + j):128 * (9 + j)],
                            hbf_src[:, BD * k:BD * (k + 1)],
                            start=(j == 0 and k == 0), stop=False)
                    nc.tensor.matmul(hgn[:, BD * j:BD * (j + 1)],
                                     dbhhn_s[0:1, 128 * j:128 * (j + 1)],
                                     ones_b[0:1, 0:BD],
                                     start=False, stop=(j == 3))

            def dec_m1bias(m1):
                for j in range(8):
                    nc.tensor.matmul(m1[:, BD * j:BD * (j + 1)],
                                     dm1b_s[0:1, 128 * j:128 * (j + 1)],
                                     ones_b[0:1, 0:BD],
                                     start=(j == 0), stop=False)

            def dec_m2bias(m2d):
                for j in range(4):
                    nc.tensor.matmul(m2d[:, BD * j:BD * (j + 1)],
                                     dm2b_s[0:1, 128 * j:128 * (j + 1)],
                                     ones_b[0:1, 0:BD],
                                     start=(j == 0), stop=False)

            rz = pA.tile([128, 512], F32, tag="A")
            hgn = pB.tile([128, 512], F32, tag="B")
            m1 = pD.tile([128, 512], F32, tag="D")
            m2d = pTR.tile([128, 512], F32, tag="TR")
            dec_m1bias(m1)
            dec_m2bias(m2d)
            # t=0: h=0, so no Whh matmuls; hgn(0) = bias only.
            for j in range(4):
                nc.tensor.matmul(hgn[:, BD * j:BD * (j + 1)],
                                 dbhhn_s[0:1, 128 * j:128 * (j + 1)],
                                 ones_b[0:1, 0:BD],
                                 start=(j == 0), stop=(j == 3))

            for t in range(hor):
                lastd = (t == hor - 1)
                # ---- y/const-side gate matmuls ----
                an = pC.tile([128, 512], F32, tag="C")
                for j in range(8):
                    nc.tensor.matmul(rz[:, BD * j:BD * (j + 1)],
                                     ycw[:, 128 * j:128 * (j + 1)], ypc[:],
                                     start=(t == 0 and j == 0),
                                     stop=(j == 7))
                for j in range(4):
                    nc.tensor.matmul(an[:, BD * j:BD * (j + 1)],
                                     ycw[:, 128 * (8 + j):128 * (9 + j)],
                                     ypc[:], start=(j == 0), stop=(j == 3))

                # ---- GRU eltwise, transposed layout [128, 4*32]; tail in
                # chunk-pair halves so M1 starts on half 0 early ----
                r_t = tp.tile([128, 4 * BD], BF16, tag="dr")
                nc.scalar.activation(r_t[:], rz[:, 0:4 * BD], AF.Sigmoid)
                z_t = tp.tile([128, 4 * BD], BF16, tag="dz")
                nc.scalar.activation(z_t[:], rz[:, 4 * BD:8 * BD], AF.Sigmoid)
                omz = tp.tile([128, 4 * BD], BF16, tag="domz")
                nc.vector.tensor_scalar(omz[:], z_t[:], -1.0, 1.0,
                                        OP.mult, OP.add)
                u_t = tp.tile([128, 4 * BD], BF16, tag="du")
                nc.gpsimd.tensor_mul(u_t[:], z_t[:], hbf[:])
                t2 = tp.tile([128, 4 * BD], BF16, tag="dt2")
                npre = tp.tile([128, 4 * BD], BF16, tag="dnp")
                n_t = tp.tile([128, 4 * BD], BF16, tag="dn")
                a_t = tp.tile([128, 4 * BD], BF16, tag="da")
                hbf_n = st.tile([128, 4 * BD], BF16, tag="hbf")
                for s in range(2):
                    dsl = slice(2 * BD * s, 2 * BD * (s + 1))
                    nc.vector.tensor_mul(t2[:, dsl], r_t[:, dsl],
                                         hgn[:, dsl])
                    nc.vector.tensor_add(npre[:, dsl], t2[:, dsl],
                                         an[:, dsl])
                    nc.scalar.activation(n_t[:, dsl], npre[:, dsl], AF.Tanh)
                    nc.vector.tensor_mul(a_t[:, dsl], omz[:, dsl],
                                         n_t[:, dsl])
                    nc.vector.tensor_add(hbf_n[:, dsl], a_t[:, dsl],
                                         u_t[:, dsl])
                    # M1 on the two ready h chunks
                    for k in (2 * s, 2 * s + 1):
                        for j in range(8):
                            nc.tensor.matmul(
                                m1[:, BD * j:BD * (j + 1)],
                                dm1_b[k][:, 128 * j:128 * (j + 1)],
                                hbf_n[:, BD * k:BD * (k + 1)],
                                start=False, stop=(k == 3 and j == 7))
                hbf = hbf_n
                hm1_s = tp.tile([128, 8 * BD], BF16, tag="dhm1")
                nc.vector.tensor_scalar_max(hm1_s[:], m1[:, 0:8 * BD], 0.0)

                # ---- M2 ----
                for k in range(8):
                    for j in range(4):
                        nc.tensor.matmul(m2d[:, BD * j:BD * (j + 1)],
                                         dm2_b[k][:, 128 * j:128 * (j + 1)],
                                         hm1_s[:, BD * k:BD * (k + 1)],
                                         start=False,
                                         stop=(k == 7 and j == 3))
                hm2_s = tp.tile([128, 4 * BD], BF16, tag="dhm2")
                nc.vector.tensor_scalar_max(hm2_s[:], m2d[:, 0:4 * BD], 0.0)

                # bias pre-issue for t+1 (fills the out-matmul wait)
                if not lastd:
                    m1_n = pD.tile([128, 512], F32, tag="D")
                    dec_m1bias(m1_n)
                    m2_n = pTR.tile([128, 512], F32, tag="TR")
                    dec_m2bias(m2_n)

                # ---- output head: y [64, 32]; do_b rides as a K=1 matmul --
                yb = pC.tile([128, 512], F32, tag="C")
                nc.tensor.matmul(yb[0:NY, 0:BD], dob_r[0:1, 0:NY],
                                 ones_b[0:1, 0:BD], start=True, stop=False)
                for k in range(4):
                    nc.tensor.matmul(yb[0:NY, 0:BD],
                                     dow_b[k][:, 0:NY],
                                     hm2_s[:, BD * k:BD * (k + 1)],
                                     start=False, stop=(k == 3))
                if not lastd:
                    # critical path: feed y back (bf16) before the f32 copy
                    nc.vector.tensor_copy(ypc[0:NY, :], yb[0:NY, 0:BD])
                y_f = tp.tile([NY, BD], F32, tag="dy")
                nc.scalar.copy(y_f[:], yb[0:NY, 0:BD])
                nc.sync.dma_start(d_out[NY * t:NY * (t + 1), :], y_f[:])
                if not lastd:
                    # pre-issue next step's h-side matmuls
                    rz_n = pA.tile([128, 512], F32, tag="A")
                    hgn_n = pB.tile([128, 512], F32, tag="B")
                    dec_whh(rz_n, hgn_n, hbf)
                    rz, hgn, m1, m2d = rz_n, hgn_n, m1_n, m2_n

    nc.compile()
    return nc


# ---------------------------------------------------------------------------
# Host-side sharding
# ---------------------------------------------------------------------------

def shard_inputs(inp, et=100, hor=60):
    f32 = np.float32

    def bf(a):
        return np.ascontiguousarray(np.asarray(a, f32).astype(BF))

    x, y = np.asarray(inp["x"], f32), np.asarray(inp["y"], f32)
    chains = [("xf", False, x), ("xb", True, x),
              ("ef", False, y), ("eb", True, y)]
    in_maps = []
    shared = {}

    def wih_aug(pre):
        wih = np.asarray(inp[pre + "_Wih"], f32)
        bih = np.asarray(inp[pre + "_bih"], f32)
        bhh = np.asarray(inp[pre + "_bhh"], f32)
        aug = np.zeros((66, G), f32)
        aug[0:64, :] = wih.T
        bias = bih.copy()
        bias[0:2 * H] += bhh[0:2 * H]
        aug[64, :] = bias
        aug[65, H:2 * H] = BIG
        return bf(aug)

    d_Wih = np.asarray(inp["d_Wih"], f32)
    d_bih = np.asarray(inp["d_bih"], f32)
    d_bhh = np.asarray(inp["d_bhh"], f32)
    dc_b = d_bih.copy()
    dc_b[0:2 * H] += d_bhh[0:2 * H]

    shared["em_w1t"] = bf(np.asarray(inp["em_W1"], f32).T)
    shared["em_w2t"] = bf(np.asarray(inp["em_W2"], f32).T)
    shared["eo_wt"] = bf(np.asarray(inp["eo_W"], f32).T)
    shared["dc_wt"] = bf(d_Wih[:, 0:2 * H].T)
    midb = np.concatenate([dc_b, np.asarray(inp["em_b1"], f32),
                           np.asarray(inp["em_b2"], f32),
                           np.asarray(inp["eo_b"], f32)])[None, :]
    shared["mid_bias"] = bf(midb)
    shared["dwy_t"] = bf(d_Wih[:, 2 * H:].T)
    shared["dwhh_t"] = bf(np.asarray(inp["d_Whh"], f32).T)
    shared["dbhhn_row"] = bf(d_bhh[None, 2 * H:])
    shared["dm_w1t"] = bf(np.asarray(inp["dm_W1"], f32).T)
    shared["dm_b1row"] = bf(np.asarray(inp["dm_b1"], f32)[None, :])
    shared["dm_w2t"] = bf(np.asarray(inp["dm_W2"], f32).T)
    shared["dm_b2row"] = bf(np.asarray(inp["dm_b2"], f32)[None, :])
    shared["do_wt"] = bf(np.asarray(inp["do_W"], f32).T)
    shared["do_brow"] = bf(np.asarray(inp["do_b"], f32)[None, :])

    for j in range(NCORE):
        chain, half = j // 2, j % 2
        pre, rev, seq = chains[chain]
        T = seq.shape[1]
        s = seq[128 * half:128 * (half + 1)]          # [128, T, 64]
        xin = np.zeros((66, et, BE), f32)
        xin[64, :, :] = 1.0
        pad = et - T
        if pad:
            xin[65, 0:pad, :] = 1.0
        order = np.arange(T)[::-1] if rev else np.arange(T)
        xin[0:64, pad:, :] = s[:, order, :].transpose(2, 1, 0)
        m = dict(shared)
        m["xin"] = bf(xin.reshape(66, et * BE))
        m["wih_aug"] = wih_aug(pre)
        m["whh_t"] = bf(np.asarray(inp[pre + "_Whh"], f32).T)
        m["bhhn_row"] = bf(np.asarray(inp[pre + "_bhh"], f32)[None, 2 * H:])
        xl = np.concatenate([x[16 * j:16 * j + 16, -1, :],
                             x[128 + 16 * j:128 + 16 * j + 16, -1, :]])
        m["xlast_t"] = bf(xl.T)
        in_maps.append(m)
    return in_maps


def unshard(results, hor=60):
    out = np.zeros((B, hor, NY), np.float32)
    for j in range(NCORE):
        o = results[j]["out"].reshape(hor, NY, BD).transpose(2, 0, 1)
        out[16 * j:16 * j + 16] = o[0:16]
        out[128 + 16 * j:128 + 16 * j + 16] = o[16:32]
    return out


_NC = None


def kernel(**inputs):
    global _NC
    from concourse.bass_utils import run_bass_kernel_spmd
    if _NC is None:
        _NC = build_nc()
    in_maps = shard_inputs(inputs)
    res = run_bass_kernel_spmd(_NC, in_maps, core_ids=list(range(NCORE)))
    return unshard(res.results)
